# revision 16
# baseline (speedup 1.0000x reference)
"""MoE Transformer kernel for 8 TRN2 NeuronCores (SPMD, batch-sharded).

Layout: per-core batch slice of 8 (B=64/8), tokens b-major (t = b*197 + s),
activations feature-major [768 -> 6x128 partitions, T free] in SBUF.
Layer 0 runs fp32 (routing exactness), layers 1-3 + experts run float32r.
MoE dispatch: top-2 via max8, PE-matmul cumsum for slot positions,
indirect-DMA scatter/gather through DRAM expert buffers (capacity 512).
"""
import sys, os, time

sys.path.insert(0, "/opt/trn_rl_repo")
import numpy as np
import ml_dtypes
from contextlib import ExitStack

import jax as _jax
_jax.config.update("jax_compilation_cache_dir", "/tmp/jaxcache")
_jax.config.update("jax_persistent_cache_min_entry_size_bytes", 0)
_jax.config.update("jax_persistent_cache_min_compile_time_secs", 0.0)

import concourse.bass as bass
import concourse.bacc as bacc
import concourse.tile as tile
from concourse import mybir

F32 = mybir.dt.float32
F32R = mybir.dt.float32r
BF16 = mybir.dt.bfloat16
EXPERT_BF16 = True
I32 = mybir.dt.int32
AF = mybir.ActivationFunctionType
ALU = mybir.AluOpType

P = 128
S, B, D, H = 197, 64, 768, 12
HD = D // H  # 64
E, K = 8, 2
LM, LS = 2, 2
DFF = 4 * D  # 3072
DT = D // P  # 6
FT = DFF // P  # 24
BL = B // 8  # 8 batches per core
T = S * BL  # 1576
NT = 13  # token tiles (13*128 = 1664)
TP = NT * P  # 1664
CH = 4  # column chunks
CW = 2 * S  # 394 columns per chunk (2 b-blocks)
CAP = 512  # expert capacity per core
TRASH = E * CAP  # 4096
XROWS = E * CAP + 1  # 4097
EPS = 1e-5

# param pack column layout (per layer), all [128, ncols] fp32
PK_LN1S, PK_LN1B = 0, DT
PK_INB = 12  # 18 cols
PK_OUTB = 30  # 6
PK_LN2S, PK_LN2B = 36, 42
PK_VB = 48  # 768 cols, replicated v-bias
PK_MOE_B1 = 816  # 8 * 24
PK_MOE_B1S = 1008  # 8 * 24
PK_MOE_B2 = 1200  # 8 * 6 -> 1248
PK_MOE_COLS = 1248
PK_FCB = 816  # 24
PK_FCBS = 840  # 24
PK_PROJB = 864  # 6 -> 870
PK_STD_COLS = 870


def build_program():
    nc = bacc.Bacc()
    dp = nc.declare_dram_parameter
    xT = dp("xT", [D, TP], F32, isOutput=False)
    ident = dp("ident", [P, P], F32, isOutput=False)
    lstrict = dp("lstrict", [P, P], F32, isOutput=False)
    allon = dp("allon", [P, P], F32, isOutput=False)
    ec8 = dp("ec8", [P, E], F32, isOutput=False)
    capmax = dp("capmax", [P, E], F32, isOutput=False)
    validc = dp("validc", [P, NT], F32, isOutput=False)
    trashc = dp("trashc", [P, NT], F32, isOutput=False)
    gate_wT = dp("gate_wT", [D, E], F32, isOutput=False)
    in_wT_m = dp("in_wT_m", [LM, D, 3 * D], F32, isOutput=False)
    out_wT_m = dp("out_wT_m", [LM, D, D], F32, isOutput=False)
    in_wT_s = dp("in_wT_s", [LS, D, 3 * D], F32, isOutput=False)
    out_wT_s = dp("out_wT_s", [LS, D, D], F32, isOutput=False)
    wedt = BF16 if EXPERT_BF16 else F32
    w1T = dp("w1T", [LM, E, D, DFF], wedt, isOutput=False)
    w2T = dp("w2T", [LM, E, DFF, D], wedt, isOutput=False)
    identb = dp("identb", [P, P], BF16, isOutput=False)
    fc_wT = dp("fc_wT", [LS, D, DFF], F32, isOutput=False)
    proj_wT = dp("proj_wT", [LS, DFF, D], F32, isOutput=False)
    mp0 = dp("mp0", [P, PK_MOE_COLS], F32, isOutput=False)
    mp1 = dp("mp1", [P, PK_MOE_COLS], F32, isOutput=False)
    sp0 = dp("sp0", [P, PK_STD_COLS], F32, isOutput=False)
    sp1 = dp("sp1", [P, PK_STD_COLS], F32, isOutput=False)
    yT = dp("yT", [D, T], F32, isOutput=True)
    logits_out = dp("logits_out", [TP, E], F32, isOutput=True)

    xydt = BF16 if EXPERT_BF16 else F32
    Xbuf = nc.dram_tensor("Xbuf", [XROWS, D], xydt)
    Ybuf = nc.dram_tensor("Ybuf", [XROWS, D], xydt)

    with tile.TileContext(nc) as tc:
        with ExitStack() as ctx:
            build_body(nc, tc, ctx, locals())
    nc.finalize()
    return nc


def build_body(nc, tc, ctx, t_):
    xT, ident, lstrict, allon = t_["xT"], t_["ident"], t_["lstrict"], t_["allon"]
    ec8, capmax, validc, trashc = t_["ec8"], t_["capmax"], t_["validc"], t_["trashc"]
    gate_wT = t_["gate_wT"]
    in_wT_m, out_wT_m = t_["in_wT_m"], t_["out_wT_m"]
    in_wT_s, out_wT_s = t_["in_wT_s"], t_["out_wT_s"]
    w1T, w2T, fc_wT, proj_wT = t_["w1T"], t_["w2T"], t_["fc_wT"], t_["proj_wT"]
    packs = [t_["mp0"], t_["mp1"], t_["sp0"], t_["sp1"]]
    yT, logits_out = t_["yT"], t_["logits_out"]
    Xbuf, Ybuf = t_["Xbuf"], t_["Ybuf"]

    consts = ctx.enter_context(tc.tile_pool(name="consts", bufs=1))
    resid = ctx.enter_context(tc.tile_pool(name="resid", bufs=1))
    route = ctx.enter_context(tc.tile_pool(name="route", bufs=1))
    parms = ctx.enter_context(tc.tile_pool(name="parms", bufs=1))

    id_f = consts.tile([P, P], F32, name="id_f", tag="id_f")
    nc.sync.dma_start(out=id_f, in_=ident[:, :])
    id_b = consts.tile([P, P], BF16, name="id_b", tag="id_b")
    nc.sync.dma_start(out=id_b, in_=t_["identb"][:, :])
    ls_f = consts.tile([P, P], F32, name="ls_f", tag="ls_f")
    nc.sync.dma_start(out=ls_f, in_=lstrict[:, :])
    on_f = consts.tile([P, P], F32, name="on_f", tag="on_f")
    nc.sync.dma_start(out=on_f, in_=allon[:, :])
    on_r = consts.tile([P, P], F32R, name="on_r", tag="on_r")
    nc.sync.dma_start(out=on_r, in_=allon[:, :].bitcast(F32R))
    ec_sb = consts.tile([P, E], F32, name="ec_sb", tag="ec_sb")
    nc.sync.dma_start(out=ec_sb, in_=ec8[:, :])
    cm_sb = consts.tile([P, E], F32, name="cm_sb", tag="cm_sb")
    nc.sync.dma_start(out=cm_sb, in_=capmax[:, :])
    va_sb = consts.tile([P, NT], F32, name="va_sb", tag="va_sb")
    nc.sync.dma_start(out=va_sb, in_=validc[:, :])
    tr_sb = consts.tile([P, NT], F32, name="tr_sb", tag="tr_sb")
    nc.sync.dma_start(out=tr_sb, in_=trashc[:, :])
    eps_r = consts.tile([1, 1], F32, name="eps_r", tag="eps_r")
    nc.vector.memset(eps_r, EPS)
    gw_sb = consts.tile([P, DT, E], F32, name="gw_sb", tag="gw_sb")
    nc.sync.dma_start(out=gw_sb, in_=gate_wT.rearrange("(a p) e -> p a e", p=P))

    # residual stream, feature-major [6][128, TP]
    x_t = []
    for dt in range(DT):
        xx = resid.tile([P, TP], F32, name=f"x{dt}", tag=f"x{dt}")
        nc.sync.dma_start(out=xx, in_=xT[dt * P:(dt + 1) * P, :])
        x_t.append(xx)

    # routing state (computed in layer 0, reused in layer 1)
    dr0_i = [route.tile([P, 1], I32, name=f"dr0_{i}", tag=f"dr0_{i}") for i in range(NT)]
    dr1_i = [route.tile([P, 1], I32, name=f"dr1_{i}", tag=f"dr1_{i}") for i in range(NT)]
    cw0_t = [route.tile([P, 1], F32, name=f"cw0_{i}", tag=f"cw0_{i}") for i in range(NT)]
    cw1_t = [route.tile([P, 1], F32, name=f"cw1_{i}", tag=f"cw1_{i}") for i in range(NT)]

    layers = [("moe", 0), ("moe", 1), ("std", 0), ("std", 1)]
    for L, (kind, li) in enumerate(layers):
        f32_mode = (L == 0)
        adt = F32 if f32_mode else F32R  # activation dtype for matmul operands
        pk_cols = PK_MOE_COLS if kind == "moe" else PK_STD_COLS
        pk = parms.tile([P, pk_cols], F32, name=f"pk{L}", tag="pk", bufs=1)
        nc.sync.dma_start(out=pk, in_=packs[L][:, :])
        in_wT = in_wT_m if kind == "moe" else in_wT_s
        out_wT = out_wT_m if kind == "moe" else out_wT_s

        attention(nc, tc, x_t, pk, in_wT, out_wT, li, adt, f32_mode, on_f, on_r, eps_r)

        if kind == "moe":
            moe_ffn(nc, tc, x_t, pk, li, adt, f32_mode, L,
                    w1T, w2T, Xbuf, Ybuf, logits_out, gw_sb,
                    id_f, id_b, ls_f, on_f, on_r, ec_sb, cm_sb, va_sb, tr_sb,
                    dr0_i, dr1_i, cw0_t, cw1_t, eps_r)
        else:
            std_ffn(nc, tc, x_t, pk, li, adt, fc_wT, proj_wT, on_f, on_r, eps_r)

    for dt in range(DT):
        nc.sync.dma_start(out=yT[dt * P:(dt + 1) * P, :], in_=x_t[dt][:, 0:T])


def layer_norm_chunk(nc, x_t, cols, W, s_col, b_col, pk, out_tiles, out_cols,
                     adt, f32_mode, on_f, on_r, sq_pool, row_pool, psp, tmp_pool,
                     eps_r=None):
    """LN over features (partitions) for a column chunk. out_tiles[dt][:, out_cols] = LN.

    Stats always run in fp32 (x tiles are fp32-produced; f32r matmul inputs must
    be produced-as-f32r). Broadcast matmuls use f32r row copies when allowed.
    """
    p_s = psp.tile([1, W], F32, name="p_s", tag="acc", bufs=2)
    p_q = psp.tile([1, W], F32, name="p_q", tag="acc", bufs=2)
    for dt in range(DT):
        sq = sq_pool.tile([P, W], F32, name="sq", tag="sq")
        nc.scalar.activation(out=sq, in_=x_t[dt][:, cols], func=AF.Square)
        nc.tensor.matmul(p_s, on_f[:, 0:1], x_t[dt][:, cols],
                         start=(dt == 0), stop=(dt == DT - 1))
        nc.tensor.matmul(p_q, on_f[:, 0:1], sq, start=(dt == 0), stop=(dt == DT - 1))
    mu = row_pool.tile([1, W], F32, name="mu", tag="mu")
    nc.scalar.mul(out=mu, in_=p_s, mul=1.0 / D)
    msq = row_pool.tile([1, W], F32, name="msq", tag="msq")
    nc.scalar.activation(out=msq, in_=mu, func=AF.Square)
    var = row_pool.tile([1, W], F32, name="var", tag="var")
    nc.scalar.mul(out=var, in_=p_q, mul=1.0 / D)
    nc.vector.tensor_sub(out=var, in0=var, in1=msq)
    sd = row_pool.tile([1, W], F32, name="sd", tag="sd")
    nc.scalar.activation(out=sd, in_=var, func=AF.Sqrt, bias=eps_r[0:1, 0:1])
    rs = row_pool.tile([1, W], F32, name="rs", tag="rs")
    nc.vector.reciprocal(out=rs, in_=sd)
    if f32_mode:
        on_x, mu_b, rs_b = on_f, mu, rs
    else:
        on_x = on_r
        mu_b = row_pool.tile([1, W], F32R, name="mu_r", tag="mu_r")
        nc.scalar.copy(out=mu_b, in_=mu)
        rs_b = row_pool.tile([1, W], F32R, name="rs_r", tag="rs_r")
        nc.scalar.copy(out=rs_b, in_=rs)
    p_mu = psp.tile([P, W], F32, name="p_mu", tag="acc", bufs=2)
    nc.tensor.matmul(p_mu, on_x[0:1, :], mu_b, start=True, stop=True)
    p_rs = psp.tile([P, W], F32, name="p_rs", tag="acc", bufs=2)
    nc.tensor.matmul(p_rs, on_x[0:1, :], rs_b, start=True, stop=True)
    for dt in range(DT):
        tmp = tmp_pool.tile([P, W], F32, name="lntmp", tag="lntmp")
        nc.vector.tensor_sub(out=tmp, in0=x_t[dt][:, cols], in1=p_mu)
        nc.vector.tensor_mul(out=tmp, in0=tmp, in1=p_rs)
        nc.vector.tensor_scalar(
            out=out_tiles[dt][:, out_cols], in0=tmp,
            scalar1=pk[:, s_col + dt:s_col + dt + 1],
            scalar2=pk[:, b_col + dt:b_col + dt + 1],
            op0=ALU.mult, op1=ALU.add)


def attention(nc, tc, x_t, pk, in_wT, out_wT, li, adt, f32_mode, on_f, on_r, eps_r):
    with ExitStack() as ctx:
        wq_p = ctx.enter_context(tc.tile_pool(name="wqkv", bufs=1))
        wo_p = ctx.enter_context(tc.tile_pool(name="wout", bufs=2))
        ln_p = ctx.enter_context(tc.tile_pool(name="ln1", bufs=2))
        qk_p = ctx.enter_context(tc.tile_pool(name="qk", bufs=1))
        v_p = ctx.enter_context(tc.tile_pool(name="vtok", bufs=4))
        o_p = ctx.enter_context(tc.tile_pool(name="oT", bufs=1))
        e_p = ctx.enter_context(tc.tile_pool(name="expt", bufs=3))
        sm_p = ctx.enter_context(tc.tile_pool(name="smrow", bufs=2))
        sq_p = ctx.enter_context(tc.tile_pool(name="sqt", bufs=1))
        row_p = ctx.enter_context(tc.tile_pool(name="lnrow", bufs=1))
        tmp_p = ctx.enter_context(tc.tile_pool(name="lntmp", bufs=2))
        psp = ctx.enter_context(tc.tile_pool(name="ps_att", bufs=1, space="PSUM"))

        wdt = F32 if f32_mode else F32R
        # qkv weight strips resident for the layer: [6][128, 2304]
        wqkv = []
        for dt in range(DT):
            w = wq_p.tile([P, 3 * D], wdt, name=f"wqkv{dt}", tag=f"wqkv{dt}")
            src = in_wT[li, dt * P:(dt + 1) * P, :]
            nc.sync.dma_start(out=w, in_=src if f32_mode else src.bitcast(F32R))
            wqkv.append(w)

        for c in range(CH):
            c0 = c * CW
            cols = slice(c0, c0 + CW)
            ln1 = [ln_p.tile([P, CW], adt, name=f"ln1_{dt}", tag=f"ln1_{dt}") for dt in range(DT)]
            layer_norm_chunk(nc, x_t, cols, CW, PK_LN1S, PK_LN1B, pk, ln1,
                             slice(0, CW), adt, f32_mode, on_f, on_r,
                             sq_p, row_p, psp, tmp_p, eps_r)
            # q,k feature-major: [12][128, 453] (padded width for rhs windows)
            QW = 394
            qk = [qk_p.tile([P, QW], adt, name=f"qk{ot}", tag=f"qk{ot}") for ot in range(12)]
            for ot in range(12):
                ps = psp.tile([P, CW], F32, name="ps_qk", tag="acc", bufs=2)
                for dt in range(DT):
                    nc.tensor.matmul(ps, wqkv[dt][:, ot * P:(ot + 1) * P], ln1[dt],
                                     start=(dt == 0), stop=(dt == DT - 1))
                nc.vector.tensor_scalar(
                    out=qk[ot][:, 0:CW], in0=ps,
                    scalar1=pk[:, PK_INB + ot:PK_INB + ot + 1], scalar2=None, op0=ALU.add)
            # v token-major per b-block: [2 tiles per b][*, 768]
            v_b = {}
            for b2 in range(2):
                for kt in range(2):
                    kh = P if kt == 0 else S - P  # 128 / 69
                    vt = v_p.tile([P, D], adt, name=f"v_{b2}_{kt}", tag="v")
                    tcol0 = b2 * S + kt * P
                    for nch in range(2):
                        ncols = slice(nch * 384, (nch + 1) * 384)
                        ps = psp.tile([P, 384], F32, name="ps_v", tag="acc", bufs=2)
                        for dt in range(DT):
                            nc.tensor.matmul(
                                ps[0:kh, :], ln1[dt][:, tcol0:tcol0 + kh],
                                wqkv[dt][:, 1536 + nch * 384:1536 + (nch + 1) * 384],
                                start=(dt == 0), stop=(dt == DT - 1))
                        nc.vector.tensor_tensor(
                            out=vt[0:kh, ncols], in0=ps[0:kh, :],
                            in1=pk[0:kh, PK_VB + nch * 384:PK_VB + (nch + 1) * 384],
                            op=ALU.add)
                    v_b[(b2, kt)] = vt
            # attention core per (b2, h)
            oT = [o_p.tile([P, CW], adt, name=f"oT{ot}", tag=f"oT{ot}") for ot in range(DT)]
            for b2 in range(2):
                q0 = b2 * S
                QS = 0 if b2 == 0 else CW - 256  # window start; q cols at QS.. fit in tile
                QO = q0 - QS  # offset of real q0 within the 256-window
                for h in range(H):
                    otq, po = h // 2, (h % 2) * HD
                    exps = []
                    for kt in range(2):
                        kh = P if kt == 0 else S - P
                        pss = psp.tile([P, 256], F32, name="ps_sc", tag="sc", bufs=4)
                        nc.tensor.matmul(
                            pss[0:kh, :],
                            qk[6 + otq][po:po + HD, q0 + kt * P:q0 + kt * P + kh],
                            qk[otq][po:po + HD, QS:QS + 256],
                            start=True, stop=True)
                        ex = e_p.tile([P, 256], adt, name="ex", tag="ex")
                        nc.scalar.activation(out=ex[0:kh, :], in_=pss[0:kh, :],
                                             func=AF.Exp, scale=0.125)
                        exps.append((ex, kh))
                    on_x = on_f if f32_mode else on_r
                    p_den = psp.tile([1, 256], F32, name="p_den", tag="acc", bufs=2)
                    for kt, (ex, kh) in enumerate(exps):
                        nc.tensor.matmul(p_den, on_x[0:kh, 0:1], ex[0:kh, :],
                                         start=(kt == 0), stop=(kt == 1))
                    rec = sm_p.tile([1, 256], adt, name="rec", tag="rec")
                    with nc.allow_low_precision("softmax recip in f32r"):
                        nc.vector.reciprocal(out=rec, in_=p_den)
                    p_bc = psp.tile([HD, 256], F32, name="p_bc", tag="ob", bufs=2)
                    nc.tensor.matmul(p_bc, on_x[0:1, 0:HD], rec, start=True, stop=True)
                    bc = sm_p.tile([HD, 256], F32, name="bc", tag="bc")
                    nc.scalar.copy(out=bc, in_=p_bc)
                    p_o = psp.tile([HD, 256], F32, name="p_o", tag="ob", bufs=2)
                    for kt, (ex, kh) in enumerate(exps):
                        nc.tensor.matmul(p_o, v_b[(b2, kt)][0:kh, h * HD:(h + 1) * HD],
                                         ex[0:kh, :], start=(kt == 0), stop=(kt == 1))
                    nc.vector.tensor_tensor(
                        out=oT[otq][po:po + HD, q0:q0 + S],
                        in0=p_o[:, QO:QO + S], in1=bc[:, QO:QO + S], op=ALU.mult)
            # out-proj + residual (in place): stream wout strips once per chunk,
            # 6 accumulators borrowed from the (now idle) sc/ob psum tags
            psop = []
            for dt in range(DT):
                tg = "sc" if dt < 4 else "ob"
                bf = 4 if dt < 4 else 2
                psop.append(psp.tile([P, CW], F32, name=f"psop{dt}", tag=tg, bufs=bf))
            for od in range(DT):
                w = wo_p.tile([P, D], wdt, name="wout", tag="wout")
                src = out_wT[li, od * P:(od + 1) * P, :]
                nc.sync.dma_start(out=w, in_=src if f32_mode else src.bitcast(F32R))
                for dt in range(DT):
                    nc.tensor.matmul(psop[dt], w[:, dt * P:(dt + 1) * P], oT[od],
                                     start=(od == 0), stop=(od == DT - 1))
            for dt in range(DT):
                tmp = tmp_p.tile([P, CW], F32, name="aotmp", tag="lntmp")
                nc.vector.tensor_scalar(
                    out=tmp, in0=psop[dt],
                    scalar1=pk[:, PK_OUTB + dt:PK_OUTB + dt + 1], scalar2=None, op0=ALU.add)
                nc.vector.tensor_add(out=x_t[dt][:, cols], in0=x_t[dt][:, cols], in1=tmp)


def moe_ffn(nc, tc, x_t, pk, li, adt, f32_mode, L, w1T, w2T, Xbuf, Ybuf,
            logits_out, gw_sb, id_f, id_b, ls_f, on_f, on_r, ec_sb, cm_sb, va_sb, tr_sb,
            dr0_i, dr1_i, cw0_t, cw1_t, eps_r):
    # --- LN2 into resident hsT (fp32; dispatch + logits consume it) ---
    with ExitStack() as ctx:
        hs_p = ctx.enter_context(tc.tile_pool(name="hsT", bufs=1))
        sq_p = ctx.enter_context(tc.tile_pool(name="sqt2", bufs=2))
        row_p = ctx.enter_context(tc.tile_pool(name="lnrow2", bufs=1))
        tmp_p = ctx.enter_context(tc.tile_pool(name="lntmp2", bufs=2))
        psp = ctx.enter_context(tc.tile_pool(name="ps_moe", bufs=1, space="PSUM"))

        hsT = [hs_p.tile([P, TP], F32, name=f"hsT{dt}", tag=f"hsT{dt}") for dt in range(DT)]
        for c in range(CH):
            cols = slice(c * CW, c * CW + CW)
            layer_norm_chunk(nc, x_t, cols, CW, PK_LN2S, PK_LN2B, pk, hsT, cols,
                             F32, True, on_f, on_r, sq_p, row_p, psp, tmp_p, eps_r)
        for dt in range(DT):
            nc.vector.memset(hsT[dt][:, T:TP], 0.0)

        if L == 0:
            # --- routing: logits, top-2, combine weights, slot positions ---
            rt_p = ctx.enter_context(tc.tile_pool(name="rt", bufs=3))
            sel_p = ctx.enter_context(tc.tile_pool(name="selp", bufs=1))
            sel_t, m1_t, m2_t = [], [], []
            for i in range(NT):
                tc0 = i * P
                ps_lg = psp.tile([P, E], F32, name="ps_lg", tag="acc", bufs=2)
                for dt in range(DT):
                    nc.tensor.matmul(ps_lg, hsT[dt][:, tc0:tc0 + P], gw_sb[:, dt, :],
                                     start=(dt == 0), stop=(dt == DT - 1))
                lg = rt_p.tile([P, E], F32, name="lg", tag="lg")
                nc.vector.tensor_copy(out=lg, in_=ps_lg)
                nc.sync.dma_start(out=logits_out[tc0:tc0 + P, :], in_=lg)
                srt = rt_p.tile([P, E], F32, name="srt", tag="srt")
                nc.vector.max(out=srt, in_=lg)
                em2 = rt_p.tile([P, 1], F32, name="em2", tag="em2")
                nc.vector.tensor_sub(out=em2, in0=srt[:, 1:2], in1=srt[:, 0:1])
                nc.scalar.activation(out=em2, in_=em2, func=AF.Exp)
                den = rt_p.tile([P, 1], F32, name="den", tag="den")
                nc.scalar.activation(out=den, in_=em2, func=AF.Identity, bias=on_f[:, 0:1])
                nc.vector.reciprocal(out=cw0_t[i], in_=den)
                nc.vector.tensor_mul(out=cw1_t[i], in0=em2, in1=cw0_t[i])
                m1 = sel_p.tile([P, E], F32, name=f"m1_{i}", tag=f"m1_{i}")
                nc.vector.tensor_scalar(out=m1, in0=lg, scalar1=srt[:, 0:1],
                                        scalar2=None, op0=ALU.is_ge)
                sel = sel_p.tile([P, E], F32, name=f"sel_{i}", tag=f"sel_{i}")
                nc.vector.tensor_scalar(out=sel, in0=lg, scalar1=srt[:, 1:2],
                                        scalar2=None, op0=ALU.is_ge)
                m2 = sel_p.tile([P, E], F32, name=f"m2_{i}", tag=f"m2_{i}")
                nc.vector.tensor_sub(out=m2, in0=sel, in1=m1)
                nc.vector.tensor_scalar_mul(out=sel, in0=sel, scalar1=va_sb[:, i:i + 1])
                sel_t.append(sel); m1_t.append(m1); m2_t.append(m2)
            for i in range(NT):
                ps_pos = psp.tile([P, E], F32, name="ps_pos", tag="acc", bufs=2)
                nc.tensor.matmul(ps_pos, ls_f, sel_t[i], start=True, stop=(i == 0))
                for j in range(i):
                    nc.tensor.matmul(ps_pos, on_f, sel_t[j], start=False, stop=(j == i - 1))
                drf = rt_p.tile([P, E], F32, name="drf", tag="drf")
                nc.vector.tensor_tensor(out=drf, in0=ps_pos, in1=ec_sb, op=ALU.add)
                nc.vector.tensor_tensor(out=drf, in0=drf, in1=cm_sb, op=ALU.min)
                for msk, dri in ((m1_t[i], dr0_i[i]), (m2_t[i], dr1_i[i])):
                    t0 = rt_p.tile([P, E], F32, name="selr", tag="selr")
                    nc.vector.tensor_mul(out=t0, in0=drf, in1=msk)
                    r0 = rt_p.tile([P, 1], F32, name="r0", tag="r0")
                    nc.vector.reduce_sum(out=r0, in_=t0, axis=mybir.AxisListType.X)
                    nc.vector.tensor_scalar(out=r0, in0=r0, scalar1=va_sb[:, i:i + 1],
                                            scalar2=None, op0=ALU.mult)
                    nc.vector.tensor_scalar(out=r0, in0=r0, scalar1=tr_sb[:, i:i + 1],
                                            scalar2=None, op0=ALU.add)
                    nc.vector.tensor_copy(out=dri, in_=r0)

        # --- dispatch: transpose hs to token-major tiles, scatter to Xbuf ---
        tok_p = ctx.enter_context(tc.tile_pool(name="tok", bufs=3))
        for i in range(NT):
            tc0 = i * P
            ht = tok_p.tile([P, D], BF16 if EXPERT_BF16 else F32, name="hstok", tag="hstok")
            for dt in range(DT):
                pt = psp.tile([P, P], F32, name="pt", tag="tr", bufs=4)
                nc.tensor.transpose(out=pt, in_=hsT[dt][:, tc0:tc0 + P], identity=id_f)
                nc.vector.tensor_copy(out=ht[:, dt * P:(dt + 1) * P], in_=pt)
            for dri in (dr0_i[i], dr1_i[i]):
                nc.gpsimd.indirect_dma_start(
                    out=Xbuf[:, :],
                    out_offset=bass.IndirectOffsetOnAxis(ap=dri[:, 0:1], axis=0),
                    in_=ht, in_offset=None)

    # --- experts ---
    with ExitStack() as ctx:
        xe_p = ctx.enter_context(tc.tile_pool(name="xe", bufs=4))
        xet_p = ctx.enter_context(tc.tile_pool(name="xet", bufs=1))
        y1_p = ctx.enter_context(tc.tile_pool(name="y1", bufs=1))
        y2_p = ctx.enter_context(tc.tile_pool(name="y2", bufs=1))
        w1_p = ctx.enter_context(tc.tile_pool(name="w1p", bufs=18))
        w2_p = ctx.enter_context(tc.tile_pool(name="w2p", bufs=6))
        sg_p = ctx.enter_context(tc.tile_pool(name="sgp", bufs=3))
        ye_p = ctx.enter_context(tc.tile_pool(name="yep", bufs=3))
        ps_e = ctx.enter_context(tc.tile_pool(name="ps_exp", bufs=1, space="PSUM"))

        for e in range(E):
            edt = BF16 if EXPERT_BF16 else F32R
            xid = id_b if EXPERT_BF16 else id_f
            xet = [xet_p.tile([P, CAP], edt, name=f"xet{dt}", tag=f"xet{dt}")
                   for dt in range(DT)]
            for i in range(CAP // P):
                xe = xe_p.tile([P, D], BF16 if EXPERT_BF16 else F32, name="xe", tag="xe")
                r0 = e * CAP + i * P
                nc.sync.dma_start(out=xe, in_=Xbuf[r0:r0 + P, :])
                for dt in range(DT):
                    pt = ps_e.tile([P, P], BF16 if EXPERT_BF16 else F32, name="pt1",
                                   tag="pmm", bufs=2)
                    nc.tensor.transpose(out=pt, in_=xe[:, dt * P:(dt + 1) * P], identity=xid)
                    nc.vector.tensor_copy(out=xet[dt][:, i * P:(i + 1) * P], in_=pt)
            y1t = [y1_p.tile([P, CAP], edt, name=f"y1t{ft}", tag=f"y1t{ft}")
                   for ft in range(FT)]
            for fg in range(6):
                w1s = []
                for dt in range(DT):
                    w = w1_p.tile([P, 4 * P], edt, name="w1s", tag="w1s")
                    src = w1T[li, e, dt * P:(dt + 1) * P, fg * 4 * P:(fg + 1) * 4 * P]
                    nc.sync.dma_start(out=w, in_=src if EXPERT_BF16 else src.bitcast(F32R))
                    w1s.append(w)
                for f4 in range(4):
                    ft = fg * 4 + f4
                    ps = ps_e.tile([P, CAP], F32, name="ps_y1", tag="pmm", bufs=2)
                    for dt in range(DT):
                        nc.tensor.matmul(ps, w1s[dt][:, f4 * P:(f4 + 1) * P], xet[dt],
                                         start=(dt == 0), stop=(dt == DT - 1))
                    b1 = pk[:, PK_MOE_B1 + e * FT + ft:PK_MOE_B1 + e * FT + ft + 1]
                    b1s = pk[:, PK_MOE_B1S + e * FT + ft:PK_MOE_B1S + e * FT + ft + 1]
                    sg = sg_p.tile([P, CAP], F32, name="sg", tag="sg")
                    nc.scalar.activation(out=sg, in_=ps, func=AF.Sigmoid,
                                         bias=b1s, scale=1.702)
                    t1 = sg_p.tile([P, CAP], F32, name="t1", tag="t1")
                    nc.vector.tensor_scalar(out=t1, in0=ps, scalar1=b1,
                                            scalar2=None, op0=ALU.add)
                    nc.vector.tensor_mul(out=y1t[ft], in0=t1, in1=sg)
            y2t = [y2_p.tile([P, CAP], F32, name=f"y2t{dt}", tag=f"y2t{dt}")
                   for dt in range(DT)]
            ps6 = [ps_e.tile([P, CAP], F32, name=f"ps6_{dt}", tag=f"ps6_{dt}", bufs=1)
                   for dt in range(DT)]
            for ft in range(FT):
                w = w2_p.tile([P, D], edt, name="w2s", tag="w2s")
                src2 = w2T[li, e, ft * P:(ft + 1) * P, :]
                nc.sync.dma_start(out=w, in_=src2 if EXPERT_BF16 else src2.bitcast(F32R))
                for dt in range(DT):
                    nc.tensor.matmul(ps6[dt], w[:, dt * P:(dt + 1) * P], y1t[ft],
                                     start=(ft == 0), stop=(ft == FT - 1))
            for dt in range(DT):
                b2 = pk[:, PK_MOE_B2 + e * DT + dt:PK_MOE_B2 + e * DT + dt + 1]
                nc.vector.tensor_scalar(out=y2t[dt], in0=ps6[dt], scalar1=b2,
                                        scalar2=None, op0=ALU.add)
            for i in range(CAP // P):
                ye = ye_p.tile([P, D], BF16 if EXPERT_BF16 else F32, name="ye", tag="ye")
                for dt in range(DT):
                    pt = ps_e.tile([P, P], F32, name="pt2", tag="pmm", bufs=2)
                    nc.tensor.transpose(out=pt, in_=y2t[dt][:, i * P:(i + 1) * P],
                                        identity=id_f)
                    nc.vector.tensor_copy(out=ye[:, dt * P:(dt + 1) * P], in_=pt)
                r0 = e * CAP + i * P
                nc.sync.dma_start(out=Ybuf[r0:r0 + P, :], in_=ye)

    # --- combine: gather per token tile, weighted sum, residual add ---
    with ExitStack() as ctx:
        g_p = ctx.enter_context(tc.tile_pool(name="gat", bufs=4))
        ps_t = ctx.enter_context(tc.tile_pool(name="ps_t2", bufs=1, space="PSUM"))
        for i in range(NT):
            tc0 = i * P
            gdt = BF16 if EXPERT_BF16 else F32
            y0 = g_p.tile([P, D], gdt, name="y0g", tag="y0g")
            nc.gpsimd.indirect_dma_start(
                out=y0, out_offset=None, in_=Ybuf[:, :],
                in_offset=bass.IndirectOffsetOnAxis(ap=dr0_i[i][:, 0:1], axis=0))
            y1g = g_p.tile([P, D], gdt, name="y1g", tag="y1g")
            nc.gpsimd.indirect_dma_start(
                out=y1g, out_offset=None, in_=Ybuf[:, :],
                in_offset=bass.IndirectOffsetOnAxis(ap=dr1_i[i][:, 0:1], axis=0))
            cmb = g_p.tile([P, D], F32, name="cmb", tag="cmb")
            nc.vector.tensor_scalar_mul(out=cmb, in0=y0, scalar1=cw0_t[i])
            t2 = g_p.tile([P, D], F32, name="t2g", tag="t2g")
            nc.vector.tensor_scalar_mul(out=t2, in0=y1g, scalar1=cw1_t[i])
            nc.vector.tensor_add(out=cmb, in0=cmb, in1=t2)
            for dt in range(DT):
                pt = ps_t.tile([P, P], F32, name="pt3", tag="tr2", bufs=4)
                nc.tensor.transpose(out=pt, in_=cmb[:, dt * P:(dt + 1) * P], identity=id_f)
                nc.vector.tensor_add(out=x_t[dt][:, tc0:tc0 + P],
                                     in0=x_t[dt][:, tc0:tc0 + P], in1=pt)


def std_ffn(nc, tc, x_t, pk, li, adt, fc_wT, proj_wT, on_f, on_r, eps_r):
    with ExitStack() as ctx:
        hs_p = ctx.enter_context(tc.tile_pool(name="hs_s", bufs=1))
        h1_p = ctx.enter_context(tc.tile_pool(name="h1", bufs=1))
        wf_p = ctx.enter_context(tc.tile_pool(name="wfc", bufs=1))
        wp_p = ctx.enter_context(tc.tile_pool(name="wproj", bufs=3))
        sg_p = ctx.enter_context(tc.tile_pool(name="sg_s", bufs=2))
        sq_p = ctx.enter_context(tc.tile_pool(name="sqt3", bufs=2))
        row_p = ctx.enter_context(tc.tile_pool(name="lnrow3", bufs=1))
        tmp_p = ctx.enter_context(tc.tile_pool(name="lntmp3", bufs=2))
        ps_st = ctx.enter_context(tc.tile_pool(name="ps_std", bufs=1, space="PSUM"))

        for c in range(CH):
            cols = slice(c * CW, c * CW + CW)
            hs = [hs_p.tile([P, CW], F32R, name=f"hs_{dt}", tag=f"hs_{dt}") for dt in range(DT)]
            layer_norm_chunk(nc, x_t, cols, CW, PK_LN2S, PK_LN2B, pk, hs,
                             slice(0, CW), F32R, False, on_f, on_r,
                             sq_p, row_p, ps_st, tmp_p, eps_r)
            h1 = [h1_p.tile([P, CW], F32R, name=f"h1_{ft}", tag=f"h1_{ft}") for ft in range(FT)]
            for half in range(2):
                wfc = []
                for dt in range(DT):
                    w = wf_p.tile([P, DFF // 2], F32R, name=f"wfc{dt}", tag=f"wfc{dt}")
                    nc.sync.dma_start(
                        out=w, in_=fc_wT[li, dt * P:(dt + 1) * P,
                                         half * (DFF // 2):(half + 1) * (DFF // 2)].bitcast(F32R))
                    wfc.append(w)
                for ft in range(half * 12, half * 12 + 12):
                    fo = ft - half * 12
                    ps = ps_st.tile([P, CW], F32, name="ps_fc", tag="acc", bufs=2)
                    for dt in range(DT):
                        nc.tensor.matmul(ps, wfc[dt][:, fo * P:(fo + 1) * P], hs[dt],
                                         start=(dt == 0), stop=(dt == DT - 1))
                    fcb = pk[:, PK_FCB + ft:PK_FCB + ft + 1]
                    fcbs = pk[:, PK_FCBS + ft:PK_FCBS + ft + 1]
                    sg = sg_p.tile([P, CW], F32, name="sg_f", tag="sg_f")
                    nc.scalar.activation(out=sg, in_=ps, func=AF.Sigmoid, bias=fcbs, scale=1.702)
                    t1 = sg_p.tile([P, CW], F32, name="t1_f", tag="t1_f")
                    nc.vector.tensor_scalar(out=t1, in0=ps, scalar1=fcb, scalar2=None, op0=ALU.add)
                    nc.vector.tensor_mul(out=h1[ft], in0=t1, in1=sg)
            ps6 = [ps_st.tile([P, CW], F32, name=f"ps6s_{dt}", tag=f"ps6s_{dt}", bufs=1)
                   for dt in range(DT)]
            for ft in range(FT):
                w = wp_p.tile([P, D], F32R, name="wpj", tag="wpj")
                nc.sync.dma_start(out=w, in_=proj_wT[li, ft * P:(ft + 1) * P, :].bitcast(F32R))
                for dt in range(DT):
                    nc.tensor.matmul(ps6[dt], w[:, dt * P:(dt + 1) * P], h1[ft],
                                     start=(ft == 0), stop=(ft == FT - 1))
            for dt in range(DT):
                tmp = tmp_p.tile([P, CW], F32, name="pjtmp", tag="lntmp3")
                nc.vector.tensor_scalar(
                    out=tmp, in0=ps6[dt],
                    scalar1=pk[:, PK_PROJB + dt:PK_PROJB + dt + 1], scalar2=None, op0=ALU.add)
                nc.vector.tensor_add(out=x_t[dt][:, cols], in0=x_t[dt][:, cols], in1=tmp)


# ---------------- host side ----------------

_CACHE = {}


def _pack_vec(pack, col, v):
    n = v.shape[0] // P
    pack[:, col:col + n] = v.reshape(n, P).T


def _build_host_constants():
    ident = np.eye(P, dtype=np.float32)
    lstrict = np.triu(np.ones((P, P), np.float32), 1)
    allon = np.ones((P, P), np.float32)
    ec8 = np.tile((np.arange(E) * CAP).astype(np.float32), (P, 1))
    capmax = np.tile(((np.arange(E) + 1) * CAP - 1).astype(np.float32), (P, 1))
    validc = np.zeros((P, NT), np.float32)
    trashc = np.zeros((P, NT), np.float32)
    for i in range(NT):
        for p in range(P):
            if i * P + p < T:
                validc[p, i] = 1.0
            else:
                trashc[p, i] = float(TRASH)
    return ident, lstrict, allon, ec8, capmax, validc, trashc


def _make_packs(inputs):
    packs = []
    for kind, li in [("moe", 0), ("moe", 1), ("std", 0), ("std", 1)]:
        pfx = "moe" if kind == "moe" else "std"
        cols = PK_MOE_COLS if kind == "moe" else PK_STD_COLS
        pack = np.zeros((P, cols), np.float32)
        _pack_vec(pack, PK_LN1S, inputs[f"{pfx}_ln1_s"][li])
        _pack_vec(pack, PK_LN1B, inputs[f"{pfx}_ln1_b"][li])
        _pack_vec(pack, PK_INB, inputs[f"{pfx}_in_b"][li])
        _pack_vec(pack, PK_OUTB, inputs[f"{pfx}_out_b"][li])
        _pack_vec(pack, PK_LN2S, inputs[f"{pfx}_ln2_s"][li])
        _pack_vec(pack, PK_LN2B, inputs[f"{pfx}_ln2_b"][li])
        pack[:, PK_VB:PK_VB + D] = np.tile(inputs[f"{pfx}_in_b"][li][2 * D:], (P, 1))
        if kind == "moe":
            for e in range(E):
                _pack_vec(pack, PK_MOE_B1 + e * FT, inputs["moe_b1"][li, e])
                _pack_vec(pack, PK_MOE_B1S + e * FT,
                          (1.702 * inputs["moe_b1"][li, e]).astype(np.float32))
                _pack_vec(pack, PK_MOE_B2 + e * DT, inputs["moe_b2"][li, e])
        else:
            _pack_vec(pack, PK_FCB, inputs["std_fc_b"][li])
            _pack_vec(pack, PK_FCBS, (1.702 * inputs["std_fc_b"][li]).astype(np.float32))
            _pack_vec(pack, PK_PROJB, inputs["std_proj_b"][li])
        packs.append(pack)
    return packs


def _prep_core_inputs(inputs, core):
    """Everything except xT is shared across cores; build shared dict once."""
    c = np.ascontiguousarray
    tr = lambda a: c(np.swapaxes(a, -1, -2))
    ident, lstrict, allon, ec8, capmax, validc, trashc = _build_host_constants()
    mp0, mp1, sp0, sp1 = _make_packs(inputs)
    wcast = (lambda a: a.astype(ml_dtypes.bfloat16)) if EXPERT_BF16 else (lambda a: a)
    shared = dict(
        ident=ident, lstrict=lstrict, allon=allon, ec8=ec8, capmax=capmax,
        validc=validc, trashc=trashc, identb=ident.astype(ml_dtypes.bfloat16),
        gate_wT=tr(inputs["moe_gate_w"][0]),
        in_wT_m=tr(inputs["moe_in_w"]), out_wT_m=tr(inputs["moe_out_w"]),
        in_wT_s=tr(inputs["std_in_w"]), out_wT_s=tr(inputs["std_out_w"]),
        w1T=wcast(tr(inputs["moe_w1"])), w2T=wcast(tr(inputs["moe_w2"])),
        fc_wT=tr(inputs["std_fc_w"]), proj_wT=tr(inputs["std_proj_w"]),
        mp0=mp0, mp1=mp1, sp0=sp0, sp1=sp1,
    )
    return shared


def _x_slice_T(x, core):
    # x [S, B, D] -> core slice b-major flat [T, D] padded to TP -> transpose
    xs = np.transpose(x[:, core * BL:(core + 1) * BL, :], (1, 0, 2)).reshape(T, D)
    xp = np.zeros((TP, D), np.float32)
    xp[:T] = xs
    return np.ascontiguousarray(xp.T)


LAST_EXEC_NS = None
N_CORES = 8


def _make_runner(nc):
    """Build a cached jitted 8-core runner (mirrors bass2jax.run_bass_via_pjrt)."""
    import jax
    from jax.experimental.shard_map import shard_map
    from jax.sharding import Mesh, PartitionSpec
    from concourse import mybir as mb
    from concourse.bass2jax import (
        _bass_exec_p, install_neuronx_cc_hook, partition_id_tensor)

    install_neuronx_cc_hook()
    partition_name = nc.partition_id_tensor.name if nc.partition_id_tensor else None
    in_names, out_names, out_avals, zero_shapes = [], [], [], []
    for alloc in nc.m.functions[0].allocations:
        if not isinstance(alloc, mb.MemoryLocationSet):
            continue
        name = alloc.memorylocations[0].name
        if alloc.kind == "ExternalInput":
            if name != partition_name:
                in_names.append(name)
        elif alloc.kind == "ExternalOutput":
            shape = tuple(alloc.tensor_shape)
            dtype = mb.dt.np(alloc.dtype)
            out_names.append(name)
            out_avals.append(jax.core.ShapedArray(shape, dtype))
            zero_shapes.append((shape, dtype))
    n_params, n_outs = len(in_names), len(out_names)
    bind_names = list(in_names) + list(out_names)
    if partition_name is not None:
        bind_names.append(partition_name)
    donate = tuple(range(n_params, n_params + n_outs))

    def _body(*args):
        operands = list(args)
        if partition_name is not None:
            operands.append(partition_id_tensor())
        outs = _bass_exec_p.bind(
            *operands,
            out_avals=tuple(out_avals),
            in_names=tuple(bind_names),
            out_names=tuple(out_names),
            lowering_input_output_aliases=(),
            sim_require_finite=True,
            sim_require_nnan=True,
            nc=nc,
        )
        return tuple(outs)

    devices = jax.devices()[:N_CORES]
    mesh = Mesh(np.asarray(devices), ("core",))
    in_specs = (PartitionSpec("core"),) * (n_params + n_outs)
    out_specs = (PartitionSpec("core"),) * n_outs
    sharded = jax.jit(
        shard_map(_body, mesh=mesh, in_specs=in_specs, out_specs=out_specs,
                  check_rep=False),
        donate_argnums=donate, keep_unused=True)

    import jax.numpy as jnp
    from jax.sharding import NamedSharding
    zsharding = NamedSharding(mesh, PartitionSpec("core"))

    @jax.jit
    def _mk_zeros():
        return tuple(
            jnp.zeros((N_CORES * s[0], *s[1:]), d) for (s, d) in zero_shapes)
    mk_zeros = jax.jit(_mk_zeros, out_shardings=tuple(zsharding for _ in zero_shapes))

    def dev_zeros():
        out = mk_zeros()
        jax.block_until_ready(out)
        return list(out)

    return dict(sharded=sharded, in_names=in_names, out_names=out_names,
                dev_zeros=dev_zeros, mesh=mesh, zsharding=zsharding)


def _get_runner():
    if "runner" not in _CACHE:
        _CACHE["nc"] = build_program()
        _CACHE["runner"] = _make_runner(_CACHE["nc"])
    return _CACHE["runner"]


def _device_inputs(inputs):
    import jax
    shared = _prep_core_inputs(inputs, 0)
    in_maps = []
    for core in range(N_CORES):
        m = dict(shared)
        m["xT"] = _x_slice_T(inputs["x"], core)
        in_maps.append(m)
    r = _get_runner()
    concat = [
        np.concatenate([np.asarray(m[name]) for m in in_maps], axis=0)
        for name in r["in_names"]
    ]
    return [jax.device_put(a, r["zsharding"]) for a in concat]


def _run(dev_in):
    import jax
    r = _get_runner()
    out = r["sharded"](*dev_in, *r["dev_zeros"]())
    jax.block_until_ready(out)
    return out


def _assemble(out_arrs):
    r = _get_runner()
    by_name = {}
    for i, name in enumerate(r["out_names"]):
        a = np.asarray(out_arrs[i])
        by_name[name] = a.reshape(N_CORES, a.shape[0] // N_CORES, *a.shape[1:])
    y = np.zeros((S, B, D), np.float32)
    fl = np.zeros((S * B, E), np.float32)
    srange = np.arange(S) * B
    for core in range(N_CORES):
        yl = by_name["yT"][core].T.reshape(BL, S, D)
        y[:, core * BL:(core + 1) * BL, :] = np.transpose(yl, (1, 0, 2))
        lg = by_name["logits_out"][core][:T].reshape(BL, S, E)
        for b in range(BL):
            fl[srange + core * BL + b] = lg[b]
    return y, fl


def kernel(**inputs):
    global LAST_EXEC_NS
    inputs = {k: np.asarray(v, dtype=np.float32) for k, v in inputs.items()}
    dev_in = _device_inputs(inputs)
    t0 = time.time()
    out = _run(dev_in)
    t1 = time.time()
    LAST_EXEC_NS = int((t1 - t0) * 1e9)
    return _assemble(out)


# revision 17
# speedup vs baseline: 26.0834x; 26.0834x over previous
"""MoE Transformer kernel for 8 TRN2 NeuronCores (SPMD, batch-sharded).

Layout: per-core batch slice of 8 (B=64/8), tokens b-major (t = b*197 + s),
activations feature-major [768 -> 6x128 partitions, T free] in SBUF.
Layer 0 runs fp32 (routing exactness), layers 1-3 + experts run float32r.
MoE dispatch: top-2 via max8, PE-matmul cumsum for slot positions,
indirect-DMA scatter/gather through DRAM expert buffers (capacity 512).
"""
import sys, os, time

sys.path.insert(0, "/opt/trn_rl_repo")
import numpy as np
import ml_dtypes
from contextlib import ExitStack

import jax as _jax
_jax.config.update("jax_compilation_cache_dir", "/tmp/jaxcache")
_jax.config.update("jax_persistent_cache_min_entry_size_bytes", 0)
_jax.config.update("jax_persistent_cache_min_compile_time_secs", 0.0)

import concourse.bass as bass
import concourse.bacc as bacc
import concourse.tile as tile
from concourse import mybir

F32 = mybir.dt.float32
F32R = mybir.dt.float32r
BF16 = mybir.dt.bfloat16
EXPERT_BF16 = False
I32 = mybir.dt.int32
AF = mybir.ActivationFunctionType
ALU = mybir.AluOpType

P = 128
S, B, D, H = 197, 64, 768, 12
HD = D // H  # 64
E, K = 8, 2
LM, LS = 2, 2
DFF = 4 * D  # 3072
DT = D // P  # 6
FT = DFF // P  # 24
BL = B // 8  # 8 batches per core
T = S * BL  # 1576
NT = 13  # token tiles (13*128 = 1664)
TP = NT * P  # 1664
CH = 4  # column chunks
CW = 2 * S  # 394 columns per chunk (2 b-blocks)
CAP = 512  # expert capacity per core
TRASH = E * CAP  # 4096
XROWS = E * CAP + 1  # 4097
EPS = 1e-5

# param pack column layout (per layer), all [128, ncols] fp32
PK_LN1S, PK_LN1B = 0, DT
PK_INB = 12  # 18 cols
PK_OUTB = 30  # 6
PK_LN2S, PK_LN2B = 36, 42
PK_VB = 48  # 768 cols, replicated v-bias
PK_MOE_B1 = 816  # 8 * 24
PK_MOE_B1S = 1008  # 8 * 24
PK_MOE_B2 = 1200  # 8 * 6 -> 1248
PK_MOE_COLS = 1248
PK_FCB = 816  # 24
PK_FCBS = 840  # 24
PK_PROJB = 864  # 6 -> 870
PK_STD_COLS = 870


def build_program():
    nc = bacc.Bacc()
    dp = nc.declare_dram_parameter
    xT = dp("xT", [D, TP], F32, isOutput=False)
    ident = dp("ident", [P, P], F32, isOutput=False)
    lstrict = dp("lstrict", [P, P], F32, isOutput=False)
    allon = dp("allon", [P, P], F32, isOutput=False)
    ec8 = dp("ec8", [P, E], F32, isOutput=False)
    capmax = dp("capmax", [P, E], F32, isOutput=False)
    validc = dp("validc", [P, NT], F32, isOutput=False)
    trashc = dp("trashc", [P, NT], F32, isOutput=False)
    gate_wT = dp("gate_wT", [D, E], F32, isOutput=False)
    in_wT_m = dp("in_wT_m", [LM, D, 3 * D], F32, isOutput=False)
    out_wT_m = dp("out_wT_m", [LM, D, D], F32, isOutput=False)
    in_wT_s = dp("in_wT_s", [LS, D, 3 * D], F32, isOutput=False)
    out_wT_s = dp("out_wT_s", [LS, D, D], F32, isOutput=False)
    wedt = BF16 if EXPERT_BF16 else F32
    w1T = dp("w1T", [LM, E, D, DFF], wedt, isOutput=False)
    w2T = dp("w2T", [LM, E, DFF, D], wedt, isOutput=False)
    identb = dp("identb", [P, P], BF16, isOutput=False)
    fc_wT = dp("fc_wT", [LS, D, DFF], F32, isOutput=False)
    proj_wT = dp("proj_wT", [LS, DFF, D], F32, isOutput=False)
    mp0 = dp("mp0", [P, PK_MOE_COLS], F32, isOutput=False)
    mp1 = dp("mp1", [P, PK_MOE_COLS], F32, isOutput=False)
    sp0 = dp("sp0", [P, PK_STD_COLS], F32, isOutput=False)
    sp1 = dp("sp1", [P, PK_STD_COLS], F32, isOutput=False)
    yT = dp("yT", [D, T], F32, isOutput=True)
    logits_out = dp("logits_out", [TP, E], F32, isOutput=True)

    xydt = BF16 if EXPERT_BF16 else F32
    Xbuf = nc.dram_tensor("Xbuf", [XROWS, D], xydt)
    Ybuf = nc.dram_tensor("Ybuf", [XROWS, D], xydt)

    with tile.TileContext(nc) as tc:
        with ExitStack() as ctx:
            build_body(nc, tc, ctx, locals())
    nc.finalize()
    return nc


def build_body(nc, tc, ctx, t_):
    xT, ident, lstrict, allon = t_["xT"], t_["ident"], t_["lstrict"], t_["allon"]
    ec8, capmax, validc, trashc = t_["ec8"], t_["capmax"], t_["validc"], t_["trashc"]
    gate_wT = t_["gate_wT"]
    in_wT_m, out_wT_m = t_["in_wT_m"], t_["out_wT_m"]
    in_wT_s, out_wT_s = t_["in_wT_s"], t_["out_wT_s"]
    w1T, w2T, fc_wT, proj_wT = t_["w1T"], t_["w2T"], t_["fc_wT"], t_["proj_wT"]
    packs = [t_["mp0"], t_["mp1"], t_["sp0"], t_["sp1"]]
    yT, logits_out = t_["yT"], t_["logits_out"]
    Xbuf, Ybuf = t_["Xbuf"], t_["Ybuf"]

    consts = ctx.enter_context(tc.tile_pool(name="consts", bufs=1))
    resid = ctx.enter_context(tc.tile_pool(name="resid", bufs=1))
    route = ctx.enter_context(tc.tile_pool(name="route", bufs=1))
    parms = ctx.enter_context(tc.tile_pool(name="parms", bufs=1))

    id_f = consts.tile([P, P], F32, name="id_f", tag="id_f")
    nc.sync.dma_start(out=id_f, in_=ident[:, :])
    id_b = consts.tile([P, P], BF16, name="id_b", tag="id_b")
    nc.sync.dma_start(out=id_b, in_=t_["identb"][:, :])
    ls_f = consts.tile([P, P], F32, name="ls_f", tag="ls_f")
    nc.sync.dma_start(out=ls_f, in_=lstrict[:, :])
    on_f = consts.tile([P, P], F32, name="on_f", tag="on_f")
    nc.sync.dma_start(out=on_f, in_=allon[:, :])
    on_r = consts.tile([P, P], F32R, name="on_r", tag="on_r")
    nc.sync.dma_start(out=on_r, in_=allon[:, :].bitcast(F32R))
    ec_sb = consts.tile([P, E], F32, name="ec_sb", tag="ec_sb")
    nc.sync.dma_start(out=ec_sb, in_=ec8[:, :])
    cm_sb = consts.tile([P, E], F32, name="cm_sb", tag="cm_sb")
    nc.sync.dma_start(out=cm_sb, in_=capmax[:, :])
    va_sb = consts.tile([P, NT], F32, name="va_sb", tag="va_sb")
    nc.sync.dma_start(out=va_sb, in_=validc[:, :])
    tr_sb = consts.tile([P, NT], F32, name="tr_sb", tag="tr_sb")
    nc.sync.dma_start(out=tr_sb, in_=trashc[:, :])
    eps_r = consts.tile([1, 1], F32, name="eps_r", tag="eps_r")
    nc.vector.memset(eps_r, EPS)
    gw_sb = consts.tile([P, DT, E], F32, name="gw_sb", tag="gw_sb")
    nc.sync.dma_start(out=gw_sb, in_=gate_wT.rearrange("(a p) e -> p a e", p=P))

    # residual stream, feature-major [6][128, TP]
    x_t = []
    for dt in range(DT):
        xx = resid.tile([P, TP], F32, name=f"x{dt}", tag=f"x{dt}")
        nc.sync.dma_start(out=xx, in_=xT[dt * P:(dt + 1) * P, :])
        x_t.append(xx)

    # routing state (computed in layer 0, reused in layer 1)
    dr0_i = [route.tile([P, 1], I32, name=f"dr0_{i}", tag=f"dr0_{i}") for i in range(NT)]
    dr1_i = [route.tile([P, 1], I32, name=f"dr1_{i}", tag=f"dr1_{i}") for i in range(NT)]
    cw0_t = [route.tile([P, 1], F32, name=f"cw0_{i}", tag=f"cw0_{i}") for i in range(NT)]
    cw1_t = [route.tile([P, 1], F32, name=f"cw1_{i}", tag=f"cw1_{i}") for i in range(NT)]

    layers = [("moe", 0), ("moe", 1), ("std", 0), ("std", 1)]
    for L, (kind, li) in enumerate(layers):
        f32_mode = (L == 0)
        adt = F32 if f32_mode else F32R  # activation dtype for matmul operands
        pk_cols = PK_MOE_COLS if kind == "moe" else PK_STD_COLS
        pk = parms.tile([P, pk_cols], F32, name=f"pk{L}", tag="pk", bufs=1)
        nc.sync.dma_start(out=pk, in_=packs[L][:, :])
        in_wT = in_wT_m if kind == "moe" else in_wT_s
        out_wT = out_wT_m if kind == "moe" else out_wT_s

        attention(nc, tc, x_t, pk, in_wT, out_wT, li, adt, f32_mode, on_f, on_r, eps_r)

        if kind == "moe":
            moe_ffn(nc, tc, x_t, pk, li, adt, f32_mode, L,
                    w1T, w2T, Xbuf, Ybuf, logits_out, gw_sb,
                    id_f, id_b, ls_f, on_f, on_r, ec_sb, cm_sb, va_sb, tr_sb,
                    dr0_i, dr1_i, cw0_t, cw1_t, eps_r)
        else:
            std_ffn(nc, tc, x_t, pk, li, adt, fc_wT, proj_wT, on_f, on_r, eps_r)

    for dt in range(DT):
        nc.sync.dma_start(out=yT[dt * P:(dt + 1) * P, :], in_=x_t[dt][:, 0:T])


def layer_norm_chunk(nc, x_t, cols, W, s_col, b_col, pk, out_tiles, out_cols,
                     adt, f32_mode, on_f, on_r, sq_pool, row_pool, psp, tmp_pool,
                     eps_r=None):
    """LN over features (partitions) for a column chunk. out_tiles[dt][:, out_cols] = LN.

    Stats always run in fp32 (x tiles are fp32-produced; f32r matmul inputs must
    be produced-as-f32r). Broadcast matmuls use f32r row copies when allowed.
    """
    p_s = psp.tile([1, W], F32, name="p_s", tag="acc", bufs=2)
    p_q = psp.tile([1, W], F32, name="p_q", tag="acc", bufs=2)
    for dt in range(DT):
        sq = sq_pool.tile([P, W], F32, name="sq", tag="sq")
        nc.scalar.activation(out=sq, in_=x_t[dt][:, cols], func=AF.Square)
        nc.tensor.matmul(p_s, on_f[:, 0:1], x_t[dt][:, cols],
                         start=(dt == 0), stop=(dt == DT - 1))
        nc.tensor.matmul(p_q, on_f[:, 0:1], sq, start=(dt == 0), stop=(dt == DT - 1))
    mu = row_pool.tile([1, W], F32, name="mu", tag="mu")
    nc.scalar.mul(out=mu, in_=p_s, mul=1.0 / D)
    msq = row_pool.tile([1, W], F32, name="msq", tag="msq")
    nc.scalar.activation(out=msq, in_=mu, func=AF.Square)
    var = row_pool.tile([1, W], F32, name="var", tag="var")
    nc.scalar.mul(out=var, in_=p_q, mul=1.0 / D)
    nc.vector.tensor_sub(out=var, in0=var, in1=msq)
    sd = row_pool.tile([1, W], F32, name="sd", tag="sd")
    nc.scalar.activation(out=sd, in_=var, func=AF.Sqrt, bias=eps_r[0:1, 0:1])
    rs = row_pool.tile([1, W], F32, name="rs", tag="rs")
    nc.vector.reciprocal(out=rs, in_=sd)
    if f32_mode:
        on_x, mu_b, rs_b = on_f, mu, rs
    else:
        on_x = on_r
        mu_b = row_pool.tile([1, W], F32R, name="mu_r", tag="mu_r")
        nc.scalar.copy(out=mu_b, in_=mu)
        rs_b = row_pool.tile([1, W], F32R, name="rs_r", tag="rs_r")
        nc.scalar.copy(out=rs_b, in_=rs)
    p_mu = psp.tile([P, W], F32, name="p_mu", tag="acc", bufs=2)
    nc.tensor.matmul(p_mu, on_x[0:1, :], mu_b, start=True, stop=True)
    p_rs = psp.tile([P, W], F32, name="p_rs", tag="acc", bufs=2)
    nc.tensor.matmul(p_rs, on_x[0:1, :], rs_b, start=True, stop=True)
    for dt in range(DT):
        tmp = tmp_pool.tile([P, W], F32, name="lntmp", tag="lntmp")
        nc.vector.tensor_sub(out=tmp, in0=x_t[dt][:, cols], in1=p_mu)
        nc.vector.tensor_mul(out=tmp, in0=tmp, in1=p_rs)
        nc.vector.tensor_scalar(
            out=out_tiles[dt][:, out_cols], in0=tmp,
            scalar1=pk[:, s_col + dt:s_col + dt + 1],
            scalar2=pk[:, b_col + dt:b_col + dt + 1],
            op0=ALU.mult, op1=ALU.add)


def attention(nc, tc, x_t, pk, in_wT, out_wT, li, adt, f32_mode, on_f, on_r, eps_r):
    with ExitStack() as ctx:
        wq_p = ctx.enter_context(tc.tile_pool(name="wqkv", bufs=1))
        wo_p = ctx.enter_context(tc.tile_pool(name="wout", bufs=2))
        ln_p = ctx.enter_context(tc.tile_pool(name="ln1", bufs=2))
        qk_p = ctx.enter_context(tc.tile_pool(name="qk", bufs=1))
        v_p = ctx.enter_context(tc.tile_pool(name="vtok", bufs=4))
        o_p = ctx.enter_context(tc.tile_pool(name="oT", bufs=1))
        e_p = ctx.enter_context(tc.tile_pool(name="expt", bufs=3))
        sm_p = ctx.enter_context(tc.tile_pool(name="smrow", bufs=2))
        sq_p = ctx.enter_context(tc.tile_pool(name="sqt", bufs=1))
        row_p = ctx.enter_context(tc.tile_pool(name="lnrow", bufs=1))
        tmp_p = ctx.enter_context(tc.tile_pool(name="lntmp", bufs=2))
        psp = ctx.enter_context(tc.tile_pool(name="ps_att", bufs=1, space="PSUM"))

        wdt = F32 if f32_mode else F32R
        # qkv weight strips resident for the layer: [6][128, 2304]
        wqkv = []
        for dt in range(DT):
            w = wq_p.tile([P, 3 * D], wdt, name=f"wqkv{dt}", tag=f"wqkv{dt}")
            src = in_wT[li, dt * P:(dt + 1) * P, :]
            nc.sync.dma_start(out=w, in_=src if f32_mode else src.bitcast(F32R))
            wqkv.append(w)

        for c in range(CH):
            c0 = c * CW
            cols = slice(c0, c0 + CW)
            ln1 = [ln_p.tile([P, CW], adt, name=f"ln1_{dt}", tag=f"ln1_{dt}") for dt in range(DT)]
            layer_norm_chunk(nc, x_t, cols, CW, PK_LN1S, PK_LN1B, pk, ln1,
                             slice(0, CW), adt, f32_mode, on_f, on_r,
                             sq_p, row_p, psp, tmp_p, eps_r)
            # q,k feature-major: [12][128, 453] (padded width for rhs windows)
            QW = 394
            qk = [qk_p.tile([P, QW], adt, name=f"qk{ot}", tag=f"qk{ot}") for ot in range(12)]
            for ot in range(12):
                ps = psp.tile([P, CW], F32, name="ps_qk", tag="acc", bufs=2)
                for dt in range(DT):
                    nc.tensor.matmul(ps, wqkv[dt][:, ot * P:(ot + 1) * P], ln1[dt],
                                     start=(dt == 0), stop=(dt == DT - 1))
                nc.vector.tensor_scalar(
                    out=qk[ot][:, 0:CW], in0=ps,
                    scalar1=pk[:, PK_INB + ot:PK_INB + ot + 1], scalar2=None, op0=ALU.add)
            # v token-major per b-block: [2 tiles per b][*, 768]
            v_b = {}
            for b2 in range(2):
                for kt in range(2):
                    kh = P if kt == 0 else S - P  # 128 / 69
                    vt = v_p.tile([P, D], adt, name=f"v_{b2}_{kt}", tag="v")
                    tcol0 = b2 * S + kt * P
                    for nch in range(2):
                        ncols = slice(nch * 384, (nch + 1) * 384)
                        ps = psp.tile([P, 384], F32, name="ps_v", tag="acc", bufs=2)
                        for dt in range(DT):
                            nc.tensor.matmul(
                                ps[0:kh, :], ln1[dt][:, tcol0:tcol0 + kh],
                                wqkv[dt][:, 1536 + nch * 384:1536 + (nch + 1) * 384],
                                start=(dt == 0), stop=(dt == DT - 1))
                        nc.vector.tensor_tensor(
                            out=vt[0:kh, ncols], in0=ps[0:kh, :],
                            in1=pk[0:kh, PK_VB + nch * 384:PK_VB + (nch + 1) * 384],
                            op=ALU.add)
                    v_b[(b2, kt)] = vt
            # attention core per (b2, h)
            oT = [o_p.tile([P, CW], adt, name=f"oT{ot}", tag=f"oT{ot}") for ot in range(DT)]
            for b2 in range(2):
                q0 = b2 * S
                QS = 0 if b2 == 0 else CW - 256  # window start; q cols at QS.. fit in tile
                QO = q0 - QS  # offset of real q0 within the 256-window
                for h in range(H):
                    otq, po = h // 2, (h % 2) * HD
                    exps = []
                    for kt in range(2):
                        kh = P if kt == 0 else S - P
                        pss = psp.tile([P, 256], F32, name="ps_sc", tag="sc", bufs=4)
                        nc.tensor.matmul(
                            pss[0:kh, :],
                            qk[6 + otq][po:po + HD, q0 + kt * P:q0 + kt * P + kh],
                            qk[otq][po:po + HD, QS:QS + 256],
                            start=True, stop=True)
                        ex = e_p.tile([P, 256], adt, name="ex", tag="ex")
                        nc.scalar.activation(out=ex[0:kh, :], in_=pss[0:kh, :],
                                             func=AF.Exp, scale=0.125)
                        exps.append((ex, kh))
                    on_x = on_f if f32_mode else on_r
                    p_den = psp.tile([1, 256], F32, name="p_den", tag="acc", bufs=2)
                    for kt, (ex, kh) in enumerate(exps):
                        nc.tensor.matmul(p_den, on_x[0:kh, 0:1], ex[0:kh, :],
                                         start=(kt == 0), stop=(kt == 1))
                    rec = sm_p.tile([1, 256], adt, name="rec", tag="rec")
                    with nc.allow_low_precision("softmax recip in f32r"):
                        nc.vector.reciprocal(out=rec, in_=p_den)
                    p_bc = psp.tile([HD, 256], F32, name="p_bc", tag="ob", bufs=2)
                    nc.tensor.matmul(p_bc, on_x[0:1, 0:HD], rec, start=True, stop=True)
                    bc = sm_p.tile([HD, 256], F32, name="bc", tag="bc")
                    nc.scalar.copy(out=bc, in_=p_bc)
                    p_o = psp.tile([HD, 256], F32, name="p_o", tag="ob", bufs=2)
                    for kt, (ex, kh) in enumerate(exps):
                        nc.tensor.matmul(p_o, v_b[(b2, kt)][0:kh, h * HD:(h + 1) * HD],
                                         ex[0:kh, :], start=(kt == 0), stop=(kt == 1))
                    nc.vector.tensor_tensor(
                        out=oT[otq][po:po + HD, q0:q0 + S],
                        in0=p_o[:, QO:QO + S], in1=bc[:, QO:QO + S], op=ALU.mult)
            # out-proj + residual (in place): stream wout strips once per chunk,
            # 6 accumulators borrowed from the (now idle) sc/ob psum tags
            psop = []
            for dt in range(DT):
                tg = "sc" if dt < 4 else "ob"
                bf = 4 if dt < 4 else 2
                psop.append(psp.tile([P, CW], F32, name=f"psop{dt}", tag=tg, bufs=bf))
            for od in range(DT):
                w = wo_p.tile([P, D], wdt, name="wout", tag="wout")
                src = out_wT[li, od * P:(od + 1) * P, :]
                nc.sync.dma_start(out=w, in_=src if f32_mode else src.bitcast(F32R))
                for dt in range(DT):
                    nc.tensor.matmul(psop[dt], w[:, dt * P:(dt + 1) * P], oT[od],
                                     start=(od == 0), stop=(od == DT - 1))
            for dt in range(DT):
                tmp = tmp_p.tile([P, CW], F32, name="aotmp", tag="lntmp")
                nc.vector.tensor_scalar(
                    out=tmp, in0=psop[dt],
                    scalar1=pk[:, PK_OUTB + dt:PK_OUTB + dt + 1], scalar2=None, op0=ALU.add)
                nc.vector.tensor_add(out=x_t[dt][:, cols], in0=x_t[dt][:, cols], in1=tmp)


def moe_ffn(nc, tc, x_t, pk, li, adt, f32_mode, L, w1T, w2T, Xbuf, Ybuf,
            logits_out, gw_sb, id_f, id_b, ls_f, on_f, on_r, ec_sb, cm_sb, va_sb, tr_sb,
            dr0_i, dr1_i, cw0_t, cw1_t, eps_r):
    # --- LN2 into resident hsT (fp32; dispatch + logits consume it) ---
    with ExitStack() as ctx:
        hs_p = ctx.enter_context(tc.tile_pool(name="hsT", bufs=1))
        sq_p = ctx.enter_context(tc.tile_pool(name="sqt2", bufs=2))
        row_p = ctx.enter_context(tc.tile_pool(name="lnrow2", bufs=1))
        tmp_p = ctx.enter_context(tc.tile_pool(name="lntmp2", bufs=2))
        psp = ctx.enter_context(tc.tile_pool(name="ps_moe", bufs=1, space="PSUM"))

        hsT = [hs_p.tile([P, TP], F32, name=f"hsT{dt}", tag=f"hsT{dt}") for dt in range(DT)]
        for c in range(CH):
            cols = slice(c * CW, c * CW + CW)
            layer_norm_chunk(nc, x_t, cols, CW, PK_LN2S, PK_LN2B, pk, hsT, cols,
                             F32, True, on_f, on_r, sq_p, row_p, psp, tmp_p, eps_r)
        for dt in range(DT):
            nc.vector.memset(hsT[dt][:, T:TP], 0.0)

        if L == 0:
            # --- routing: logits, top-2, combine weights, slot positions ---
            rt_p = ctx.enter_context(tc.tile_pool(name="rt", bufs=3))
            sel_p = ctx.enter_context(tc.tile_pool(name="selp", bufs=1))
            sel_t, m1_t, m2_t = [], [], []
            for i in range(NT):
                tc0 = i * P
                ps_lg = psp.tile([P, E], F32, name="ps_lg", tag="acc", bufs=2)
                for dt in range(DT):
                    nc.tensor.matmul(ps_lg, hsT[dt][:, tc0:tc0 + P], gw_sb[:, dt, :],
                                     start=(dt == 0), stop=(dt == DT - 1))
                lg = rt_p.tile([P, E], F32, name="lg", tag="lg")
                nc.vector.tensor_copy(out=lg, in_=ps_lg)
                nc.sync.dma_start(out=logits_out[tc0:tc0 + P, :], in_=lg)
                srt = rt_p.tile([P, E], F32, name="srt", tag="srt")
                nc.vector.max(out=srt, in_=lg)
                em2 = rt_p.tile([P, 1], F32, name="em2", tag="em2")
                nc.vector.tensor_sub(out=em2, in0=srt[:, 1:2], in1=srt[:, 0:1])
                nc.scalar.activation(out=em2, in_=em2, func=AF.Exp)
                den = rt_p.tile([P, 1], F32, name="den", tag="den")
                nc.scalar.activation(out=den, in_=em2, func=AF.Identity, bias=on_f[:, 0:1])
                nc.vector.reciprocal(out=cw0_t[i], in_=den)
                nc.vector.tensor_mul(out=cw1_t[i], in0=em2, in1=cw0_t[i])
                m1 = sel_p.tile([P, E], F32, name=f"m1_{i}", tag=f"m1_{i}")
                nc.vector.tensor_scalar(out=m1, in0=lg, scalar1=srt[:, 0:1],
                                        scalar2=None, op0=ALU.is_ge)
                sel = sel_p.tile([P, E], F32, name=f"sel_{i}", tag=f"sel_{i}")
                nc.vector.tensor_scalar(out=sel, in0=lg, scalar1=srt[:, 1:2],
                                        scalar2=None, op0=ALU.is_ge)
                m2 = sel_p.tile([P, E], F32, name=f"m2_{i}", tag=f"m2_{i}")
                nc.vector.tensor_sub(out=m2, in0=sel, in1=m1)
                nc.vector.tensor_scalar_mul(out=sel, in0=sel, scalar1=va_sb[:, i:i + 1])
                sel_t.append(sel); m1_t.append(m1); m2_t.append(m2)
            for i in range(NT):
                ps_pos = psp.tile([P, E], F32, name="ps_pos", tag="acc", bufs=2)
                nc.tensor.matmul(ps_pos, ls_f, sel_t[i], start=True, stop=(i == 0))
                for j in range(i):
                    nc.tensor.matmul(ps_pos, on_f, sel_t[j], start=False, stop=(j == i - 1))
                drf = rt_p.tile([P, E], F32, name="drf", tag="drf")
                nc.vector.tensor_tensor(out=drf, in0=ps_pos, in1=ec_sb, op=ALU.add)
                nc.vector.tensor_tensor(out=drf, in0=drf, in1=cm_sb, op=ALU.min)
                for msk, dri in ((m1_t[i], dr0_i[i]), (m2_t[i], dr1_i[i])):
                    t0 = rt_p.tile([P, E], F32, name="selr", tag="selr")
                    nc.vector.tensor_mul(out=t0, in0=drf, in1=msk)
                    r0 = rt_p.tile([P, 1], F32, name="r0", tag="r0")
                    nc.vector.reduce_sum(out=r0, in_=t0, axis=mybir.AxisListType.X)
                    nc.vector.tensor_scalar(out=r0, in0=r0, scalar1=va_sb[:, i:i + 1],
                                            scalar2=None, op0=ALU.mult)
                    nc.vector.tensor_scalar(out=r0, in0=r0, scalar1=tr_sb[:, i:i + 1],
                                            scalar2=None, op0=ALU.add)
                    nc.vector.tensor_copy(out=dri, in_=r0)

        # --- dispatch: transpose hs to token-major tiles, scatter to Xbuf ---
        tok_p = ctx.enter_context(tc.tile_pool(name="tok", bufs=3))
        for i in range(NT):
            tc0 = i * P
            ht = tok_p.tile([P, D], BF16 if EXPERT_BF16 else F32, name="hstok", tag="hstok")
            for dt in range(DT):
                pt = psp.tile([P, P], F32, name="pt", tag="tr", bufs=4)
                nc.tensor.transpose(out=pt, in_=hsT[dt][:, tc0:tc0 + P], identity=id_f)
                nc.vector.tensor_copy(out=ht[:, dt * P:(dt + 1) * P], in_=pt)
            for dri in (dr0_i[i], dr1_i[i]):
                nc.gpsimd.indirect_dma_start(
                    out=Xbuf[:, :],
                    out_offset=bass.IndirectOffsetOnAxis(ap=dri[:, 0:1], axis=0),
                    in_=ht, in_offset=None)

    # --- experts ---
    with ExitStack() as ctx:
        xe_p = ctx.enter_context(tc.tile_pool(name="xe", bufs=4))
        xet_p = ctx.enter_context(tc.tile_pool(name="xet", bufs=1))
        y1_p = ctx.enter_context(tc.tile_pool(name="y1", bufs=1))
        y2_p = ctx.enter_context(tc.tile_pool(name="y2", bufs=1))
        w1_p = ctx.enter_context(tc.tile_pool(name="w1p", bufs=18))
        w2_p = ctx.enter_context(tc.tile_pool(name="w2p", bufs=6))
        sg_p = ctx.enter_context(tc.tile_pool(name="sgp", bufs=3))
        ye_p = ctx.enter_context(tc.tile_pool(name="yep", bufs=3))
        ps_e = ctx.enter_context(tc.tile_pool(name="ps_exp", bufs=1, space="PSUM"))

        for e in range(E):
            edt = BF16 if EXPERT_BF16 else F32R
            xid = id_b if EXPERT_BF16 else id_f
            xet = [xet_p.tile([P, CAP], edt, name=f"xet{dt}", tag=f"xet{dt}")
                   for dt in range(DT)]
            for i in range(CAP // P):
                xe = xe_p.tile([P, D], BF16 if EXPERT_BF16 else F32, name="xe", tag="xe")
                r0 = e * CAP + i * P
                nc.sync.dma_start(out=xe, in_=Xbuf[r0:r0 + P, :])
                for dt in range(DT):
                    pt = ps_e.tile([P, P], BF16 if EXPERT_BF16 else F32, name="pt1",
                                   tag="pmm", bufs=2)
                    nc.tensor.transpose(out=pt, in_=xe[:, dt * P:(dt + 1) * P], identity=xid)
                    nc.vector.tensor_copy(out=xet[dt][:, i * P:(i + 1) * P], in_=pt)
            y1t = [y1_p.tile([P, CAP], edt, name=f"y1t{ft}", tag=f"y1t{ft}")
                   for ft in range(FT)]
            for fg in range(6):
                w1s = []
                for dt in range(DT):
                    w = w1_p.tile([P, 4 * P], edt, name="w1s", tag="w1s")
                    src = w1T[li, e, dt * P:(dt + 1) * P, fg * 4 * P:(fg + 1) * 4 * P]
                    nc.sync.dma_start(out=w, in_=src if EXPERT_BF16 else src.bitcast(F32R))
                    w1s.append(w)
                for f4 in range(4):
                    ft = fg * 4 + f4
                    ps = ps_e.tile([P, CAP], F32, name="ps_y1", tag="pmm", bufs=2)
                    for dt in range(DT):
                        nc.tensor.matmul(ps, w1s[dt][:, f4 * P:(f4 + 1) * P], xet[dt],
                                         start=(dt == 0), stop=(dt == DT - 1))
                    b1 = pk[:, PK_MOE_B1 + e * FT + ft:PK_MOE_B1 + e * FT + ft + 1]
                    b1s = pk[:, PK_MOE_B1S + e * FT + ft:PK_MOE_B1S + e * FT + ft + 1]
                    sg = sg_p.tile([P, CAP], F32, name="sg", tag="sg")
                    nc.scalar.activation(out=sg, in_=ps, func=AF.Sigmoid,
                                         bias=b1s, scale=1.702)
                    t1 = sg_p.tile([P, CAP], F32, name="t1", tag="t1")
                    nc.vector.tensor_scalar(out=t1, in0=ps, scalar1=b1,
                                            scalar2=None, op0=ALU.add)
                    nc.vector.tensor_mul(out=y1t[ft], in0=t1, in1=sg)
            y2t = [y2_p.tile([P, CAP], F32, name=f"y2t{dt}", tag=f"y2t{dt}")
                   for dt in range(DT)]
            ps6 = [ps_e.tile([P, CAP], F32, name=f"ps6_{dt}", tag=f"ps6_{dt}", bufs=1)
                   for dt in range(DT)]
            for ft in range(FT):
                w = w2_p.tile([P, D], edt, name="w2s", tag="w2s")
                src2 = w2T[li, e, ft * P:(ft + 1) * P, :]
                nc.sync.dma_start(out=w, in_=src2 if EXPERT_BF16 else src2.bitcast(F32R))
                for dt in range(DT):
                    nc.tensor.matmul(ps6[dt], w[:, dt * P:(dt + 1) * P], y1t[ft],
                                     start=(ft == 0), stop=(ft == FT - 1))
            for dt in range(DT):
                b2 = pk[:, PK_MOE_B2 + e * DT + dt:PK_MOE_B2 + e * DT + dt + 1]
                nc.vector.tensor_scalar(out=y2t[dt], in0=ps6[dt], scalar1=b2,
                                        scalar2=None, op0=ALU.add)
            for i in range(CAP // P):
                ye = ye_p.tile([P, D], BF16 if EXPERT_BF16 else F32, name="ye", tag="ye")
                for dt in range(DT):
                    pt = ps_e.tile([P, P], F32, name="pt2", tag="pmm", bufs=2)
                    nc.tensor.transpose(out=pt, in_=y2t[dt][:, i * P:(i + 1) * P],
                                        identity=id_f)
                    nc.vector.tensor_copy(out=ye[:, dt * P:(dt + 1) * P], in_=pt)
                r0 = e * CAP + i * P
                nc.sync.dma_start(out=Ybuf[r0:r0 + P, :], in_=ye)

    # --- combine: gather per token tile, weighted sum, residual add ---
    with ExitStack() as ctx:
        g_p = ctx.enter_context(tc.tile_pool(name="gat", bufs=4))
        ps_t = ctx.enter_context(tc.tile_pool(name="ps_t2", bufs=1, space="PSUM"))
        for i in range(NT):
            tc0 = i * P
            gdt = BF16 if EXPERT_BF16 else F32
            y0 = g_p.tile([P, D], gdt, name="y0g", tag="y0g")
            nc.gpsimd.indirect_dma_start(
                out=y0, out_offset=None, in_=Ybuf[:, :],
                in_offset=bass.IndirectOffsetOnAxis(ap=dr0_i[i][:, 0:1], axis=0))
            y1g = g_p.tile([P, D], gdt, name="y1g", tag="y1g")
            nc.gpsimd.indirect_dma_start(
                out=y1g, out_offset=None, in_=Ybuf[:, :],
                in_offset=bass.IndirectOffsetOnAxis(ap=dr1_i[i][:, 0:1], axis=0))
            cmb = g_p.tile([P, D], F32, name="cmb", tag="cmb")
            nc.vector.tensor_scalar_mul(out=cmb, in0=y0, scalar1=cw0_t[i])
            t2 = g_p.tile([P, D], F32, name="t2g", tag="t2g")
            nc.vector.tensor_scalar_mul(out=t2, in0=y1g, scalar1=cw1_t[i])
            nc.vector.tensor_add(out=cmb, in0=cmb, in1=t2)
            for dt in range(DT):
                pt = ps_t.tile([P, P], F32, name="pt3", tag="tr2", bufs=4)
                nc.tensor.transpose(out=pt, in_=cmb[:, dt * P:(dt + 1) * P], identity=id_f)
                nc.vector.tensor_add(out=x_t[dt][:, tc0:tc0 + P],
                                     in0=x_t[dt][:, tc0:tc0 + P], in1=pt)


def std_ffn(nc, tc, x_t, pk, li, adt, fc_wT, proj_wT, on_f, on_r, eps_r):
    with ExitStack() as ctx:
        hs_p = ctx.enter_context(tc.tile_pool(name="hs_s", bufs=1))
        h1_p = ctx.enter_context(tc.tile_pool(name="h1", bufs=1))
        wf_p = ctx.enter_context(tc.tile_pool(name="wfc", bufs=1))
        wp_p = ctx.enter_context(tc.tile_pool(name="wproj", bufs=3))
        sg_p = ctx.enter_context(tc.tile_pool(name="sg_s", bufs=2))
        sq_p = ctx.enter_context(tc.tile_pool(name="sqt3", bufs=2))
        row_p = ctx.enter_context(tc.tile_pool(name="lnrow3", bufs=1))
        tmp_p = ctx.enter_context(tc.tile_pool(name="lntmp3", bufs=2))
        ps_st = ctx.enter_context(tc.tile_pool(name="ps_std", bufs=1, space="PSUM"))

        for c in range(CH):
            cols = slice(c * CW, c * CW + CW)
            hs = [hs_p.tile([P, CW], F32R, name=f"hs_{dt}", tag=f"hs_{dt}") for dt in range(DT)]
            layer_norm_chunk(nc, x_t, cols, CW, PK_LN2S, PK_LN2B, pk, hs,
                             slice(0, CW), F32R, False, on_f, on_r,
                             sq_p, row_p, ps_st, tmp_p, eps_r)
            h1 = [h1_p.tile([P, CW], F32R, name=f"h1_{ft}", tag=f"h1_{ft}") for ft in range(FT)]
            for half in range(2):
                wfc = []
                for dt in range(DT):
                    w = wf_p.tile([P, DFF // 2], F32R, name=f"wfc{dt}", tag=f"wfc{dt}")
                    nc.sync.dma_start(
                        out=w, in_=fc_wT[li, dt * P:(dt + 1) * P,
                                         half * (DFF // 2):(half + 1) * (DFF // 2)].bitcast(F32R))
                    wfc.append(w)
                for ft in range(half * 12, half * 12 + 12):
                    fo = ft - half * 12
                    ps = ps_st.tile([P, CW], F32, name="ps_fc", tag="acc", bufs=2)
                    for dt in range(DT):
                        nc.tensor.matmul(ps, wfc[dt][:, fo * P:(fo + 1) * P], hs[dt],
                                         start=(dt == 0), stop=(dt == DT - 1))
                    fcb = pk[:, PK_FCB + ft:PK_FCB + ft + 1]
                    fcbs = pk[:, PK_FCBS + ft:PK_FCBS + ft + 1]
                    sg = sg_p.tile([P, CW], F32, name="sg_f", tag="sg_f")
                    nc.scalar.activation(out=sg, in_=ps, func=AF.Sigmoid, bias=fcbs, scale=1.702)
                    t1 = sg_p.tile([P, CW], F32, name="t1_f", tag="t1_f")
                    nc.vector.tensor_scalar(out=t1, in0=ps, scalar1=fcb, scalar2=None, op0=ALU.add)
                    nc.vector.tensor_mul(out=h1[ft], in0=t1, in1=sg)
            ps6 = [ps_st.tile([P, CW], F32, name=f"ps6s_{dt}", tag=f"ps6s_{dt}", bufs=1)
                   for dt in range(DT)]
            for ft in range(FT):
                w = wp_p.tile([P, D], F32R, name="wpj", tag="wpj")
                nc.sync.dma_start(out=w, in_=proj_wT[li, ft * P:(ft + 1) * P, :].bitcast(F32R))
                for dt in range(DT):
                    nc.tensor.matmul(ps6[dt], w[:, dt * P:(dt + 1) * P], h1[ft],
                                     start=(ft == 0), stop=(ft == FT - 1))
            for dt in range(DT):
                tmp = tmp_p.tile([P, CW], F32, name="pjtmp", tag="lntmp3")
                nc.vector.tensor_scalar(
                    out=tmp, in0=ps6[dt],
                    scalar1=pk[:, PK_PROJB + dt:PK_PROJB + dt + 1], scalar2=None, op0=ALU.add)
                nc.vector.tensor_add(out=x_t[dt][:, cols], in0=x_t[dt][:, cols], in1=tmp)


# ---------------- host side ----------------

_CACHE = {}


def _pack_vec(pack, col, v):
    n = v.shape[0] // P
    pack[:, col:col + n] = v.reshape(n, P).T


def _build_host_constants():
    ident = np.eye(P, dtype=np.float32)
    lstrict = np.triu(np.ones((P, P), np.float32), 1)
    allon = np.ones((P, P), np.float32)
    ec8 = np.tile((np.arange(E) * CAP).astype(np.float32), (P, 1))
    capmax = np.tile(((np.arange(E) + 1) * CAP - 1).astype(np.float32), (P, 1))
    validc = np.zeros((P, NT), np.float32)
    trashc = np.zeros((P, NT), np.float32)
    for i in range(NT):
        for p in range(P):
            if i * P + p < T:
                validc[p, i] = 1.0
            else:
                trashc[p, i] = float(TRASH)
    return ident, lstrict, allon, ec8, capmax, validc, trashc


def _make_packs(inputs):
    packs = []
    for kind, li in [("moe", 0), ("moe", 1), ("std", 0), ("std", 1)]:
        pfx = "moe" if kind == "moe" else "std"
        cols = PK_MOE_COLS if kind == "moe" else PK_STD_COLS
        pack = np.zeros((P, cols), np.float32)
        _pack_vec(pack, PK_LN1S, inputs[f"{pfx}_ln1_s"][li])
        _pack_vec(pack, PK_LN1B, inputs[f"{pfx}_ln1_b"][li])
        _pack_vec(pack, PK_INB, inputs[f"{pfx}_in_b"][li])
        _pack_vec(pack, PK_OUTB, inputs[f"{pfx}_out_b"][li])
        _pack_vec(pack, PK_LN2S, inputs[f"{pfx}_ln2_s"][li])
        _pack_vec(pack, PK_LN2B, inputs[f"{pfx}_ln2_b"][li])
        pack[:, PK_VB:PK_VB + D] = np.tile(inputs[f"{pfx}_in_b"][li][2 * D:], (P, 1))
        if kind == "moe":
            for e in range(E):
                _pack_vec(pack, PK_MOE_B1 + e * FT, inputs["moe_b1"][li, e])
                _pack_vec(pack, PK_MOE_B1S + e * FT,
                          (1.702 * inputs["moe_b1"][li, e]).astype(np.float32))
                _pack_vec(pack, PK_MOE_B2 + e * DT, inputs["moe_b2"][li, e])
        else:
            _pack_vec(pack, PK_FCB, inputs["std_fc_b"][li])
            _pack_vec(pack, PK_FCBS, (1.702 * inputs["std_fc_b"][li]).astype(np.float32))
            _pack_vec(pack, PK_PROJB, inputs["std_proj_b"][li])
        packs.append(pack)
    return packs


def _prep_core_inputs(inputs, core):
    """Everything except xT is shared across cores; build shared dict once."""
    c = np.ascontiguousarray
    tr = lambda a: c(np.swapaxes(a, -1, -2))
    ident, lstrict, allon, ec8, capmax, validc, trashc = _build_host_constants()
    mp0, mp1, sp0, sp1 = _make_packs(inputs)
    wcast = (lambda a: a.astype(ml_dtypes.bfloat16)) if EXPERT_BF16 else (lambda a: a)
    shared = dict(
        ident=ident, lstrict=lstrict, allon=allon, ec8=ec8, capmax=capmax,
        validc=validc, trashc=trashc, identb=ident.astype(ml_dtypes.bfloat16),
        gate_wT=tr(inputs["moe_gate_w"][0]),
        in_wT_m=tr(inputs["moe_in_w"]), out_wT_m=tr(inputs["moe_out_w"]),
        in_wT_s=tr(inputs["std_in_w"]), out_wT_s=tr(inputs["std_out_w"]),
        w1T=wcast(tr(inputs["moe_w1"])), w2T=wcast(tr(inputs["moe_w2"])),
        fc_wT=tr(inputs["std_fc_w"]), proj_wT=tr(inputs["std_proj_w"]),
        mp0=mp0, mp1=mp1, sp0=sp0, sp1=sp1,
    )
    return shared


def _x_slice_T(x, core):
    # x [S, B, D] -> core slice b-major flat [T, D] padded to TP -> transpose
    xs = np.transpose(x[:, core * BL:(core + 1) * BL, :], (1, 0, 2)).reshape(T, D)
    xp = np.zeros((TP, D), np.float32)
    xp[:T] = xs
    return np.ascontiguousarray(xp.T)


LAST_EXEC_NS = None
N_CORES = 8


def _make_runner(nc):
    """Build a cached jitted 8-core runner (mirrors bass2jax.run_bass_via_pjrt)."""
    import jax
    from jax.experimental.shard_map import shard_map
    from jax.sharding import Mesh, PartitionSpec
    from concourse import mybir as mb
    from concourse.bass2jax import (
        _bass_exec_p, install_neuronx_cc_hook, partition_id_tensor)

    install_neuronx_cc_hook()
    partition_name = nc.partition_id_tensor.name if nc.partition_id_tensor else None
    in_names, out_names, out_avals, zero_shapes = [], [], [], []
    for alloc in nc.m.functions[0].allocations:
        if not isinstance(alloc, mb.MemoryLocationSet):
            continue
        name = alloc.memorylocations[0].name
        if alloc.kind == "ExternalInput":
            if name != partition_name:
                in_names.append(name)
        elif alloc.kind == "ExternalOutput":
            shape = tuple(alloc.tensor_shape)
            dtype = mb.dt.np(alloc.dtype)
            out_names.append(name)
            out_avals.append(jax.core.ShapedArray(shape, dtype))
            zero_shapes.append((shape, dtype))
    n_params, n_outs = len(in_names), len(out_names)
    bind_names = list(in_names) + list(out_names)
    if partition_name is not None:
        bind_names.append(partition_name)
    donate = tuple(range(n_params, n_params + n_outs))

    def _body(*args):
        operands = list(args)
        if partition_name is not None:
            operands.append(partition_id_tensor())
        outs = _bass_exec_p.bind(
            *operands,
            out_avals=tuple(out_avals),
            in_names=tuple(bind_names),
            out_names=tuple(out_names),
            lowering_input_output_aliases=(),
            sim_require_finite=True,
            sim_require_nnan=True,
            nc=nc,
        )
        return tuple(outs)

    devices = jax.devices()[:N_CORES]
    mesh = Mesh(np.asarray(devices), ("core",))
    in_specs = (PartitionSpec("core"),) * (n_params + n_outs)
    out_specs = (PartitionSpec("core"),) * n_outs
    sharded = jax.jit(
        shard_map(_body, mesh=mesh, in_specs=in_specs, out_specs=out_specs,
                  check_rep=False),
        donate_argnums=donate, keep_unused=True)

    import jax.numpy as jnp
    from jax.sharding import NamedSharding
    zsharding = NamedSharding(mesh, PartitionSpec("core"))

    @jax.jit
    def _mk_zeros():
        return tuple(
            jnp.zeros((N_CORES * s[0], *s[1:]), d) for (s, d) in zero_shapes)
    mk_zeros = jax.jit(_mk_zeros, out_shardings=tuple(zsharding for _ in zero_shapes))

    def dev_zeros():
        out = mk_zeros()
        jax.block_until_ready(out)
        return list(out)

    return dict(sharded=sharded, in_names=in_names, out_names=out_names,
                dev_zeros=dev_zeros, mesh=mesh, zsharding=zsharding)


def _get_runner():
    if "runner" not in _CACHE:
        _CACHE["nc"] = build_program()
        _CACHE["runner"] = _make_runner(_CACHE["nc"])
    return _CACHE["runner"]


def _device_inputs(inputs):
    import jax
    shared = _prep_core_inputs(inputs, 0)
    in_maps = []
    for core in range(N_CORES):
        m = dict(shared)
        m["xT"] = _x_slice_T(inputs["x"], core)
        in_maps.append(m)
    r = _get_runner()
    concat = [
        np.concatenate([np.asarray(m[name]) for m in in_maps], axis=0)
        for name in r["in_names"]
    ]
    return [jax.device_put(a, r["zsharding"]) for a in concat]


def _run(dev_in):
    import jax
    r = _get_runner()
    out = r["sharded"](*dev_in, *r["dev_zeros"]())
    jax.block_until_ready(out)
    return out


def _assemble(out_arrs):
    r = _get_runner()
    by_name = {}
    for i, name in enumerate(r["out_names"]):
        a = np.asarray(out_arrs[i])
        by_name[name] = a.reshape(N_CORES, a.shape[0] // N_CORES, *a.shape[1:])
    y = np.zeros((S, B, D), np.float32)
    fl = np.zeros((S * B, E), np.float32)
    srange = np.arange(S) * B
    for core in range(N_CORES):
        yl = by_name["yT"][core].T.reshape(BL, S, D)
        y[:, core * BL:(core + 1) * BL, :] = np.transpose(yl, (1, 0, 2))
        lg = by_name["logits_out"][core][:T].reshape(BL, S, E)
        for b in range(BL):
            fl[srange + core * BL + b] = lg[b]
    return y, fl


def kernel(**inputs):
    global LAST_EXEC_NS
    inputs = {k: np.asarray(v, dtype=np.float32) for k, v in inputs.items()}
    dev_in = _device_inputs(inputs)
    t0 = time.time()
    out = _run(dev_in)
    t1 = time.time()
    LAST_EXEC_NS = int((t1 - t0) * 1e9)
    return _assemble(out)


# revision 21
# speedup vs baseline: 26.9915x; 1.0348x over previous
"""MoE Transformer kernel for 8 TRN2 NeuronCores (SPMD, batch-sharded).

Layout: per-core batch slice of 8 (B=64/8), tokens b-major (t = b*197 + s),
activations feature-major [768 -> 6x128 partitions, T free] in SBUF.
Layer 0 runs fp32 (routing exactness), layers 1-3 + experts run float32r.
MoE dispatch: top-2 via max8, PE-matmul cumsum for slot positions,
indirect-DMA scatter/gather through DRAM expert buffers (capacity 512).
"""
import sys, os, time

sys.path.insert(0, "/opt/trn_rl_repo")
import numpy as np
import ml_dtypes
from contextlib import ExitStack

import jax as _jax
_jax.config.update("jax_compilation_cache_dir", "/tmp/jaxcache")
_jax.config.update("jax_persistent_cache_min_entry_size_bytes", 0)
_jax.config.update("jax_persistent_cache_min_compile_time_secs", 0.0)

import concourse.bass as bass
import concourse.bacc as bacc
import concourse.tile as tile
from concourse import mybir

F32 = mybir.dt.float32
F32R = mybir.dt.float32r
BF16 = mybir.dt.bfloat16
EXPERT_BF16 = False
I32 = mybir.dt.int32
AF = mybir.ActivationFunctionType
ALU = mybir.AluOpType

P = 128
S, B, D, H = 197, 64, 768, 12
HD = D // H  # 64
E, K = 8, 2
LM, LS = 2, 2
DFF = 4 * D  # 3072
DT = D // P  # 6
FT = DFF // P  # 24
BL = B // 8  # 8 batches per core
T = S * BL  # 1576
NT = 13  # token tiles (13*128 = 1664)
TP = NT * P  # 1664
CH = 4  # column chunks
CW = 2 * S  # 394 columns per chunk (2 b-blocks)
CAP = 512  # expert capacity per core
TRASH = E * CAP  # 4096
XROWS = E * CAP + 1  # 4097
EPS = 1e-5

# param pack column layout (per layer), all [128, ncols] fp32
PK_LN1S, PK_LN1B = 0, DT
PK_INB = 12  # 18 cols
PK_OUTB = 30  # 6
PK_LN2S, PK_LN2B = 36, 42
PK_VB = 48  # 768 cols, replicated v-bias
PK_MOE_B1 = 816  # 8 * 24
PK_MOE_B1S = 1008  # 8 * 24
PK_MOE_B2 = 1200  # 8 * 6 -> 1248
PK_MOE_COLS = 1248
PK_FCB = 816  # 24
PK_FCBS = 840  # 24
PK_PROJB = 864  # 6 -> 870
PK_STD_COLS = 870


def build_program():
    nc = bacc.Bacc()
    dp = nc.declare_dram_parameter
    xT = dp("xT", [D, TP], F32, isOutput=False)
    ident = dp("ident", [P, P], F32, isOutput=False)
    lstrict = dp("lstrict", [P, P], F32, isOutput=False)
    allon = dp("allon", [P, P], F32, isOutput=False)
    ec8 = dp("ec8", [P, E], F32, isOutput=False)
    capmax = dp("capmax", [P, E], F32, isOutput=False)
    validc = dp("validc", [P, NT], F32, isOutput=False)
    trashc = dp("trashc", [P, NT], F32, isOutput=False)
    gate_wT = dp("gate_wT", [D, E], F32, isOutput=False)
    in_wT_m = dp("in_wT_m", [LM, D, 3 * D], F32, isOutput=False)
    out_wT_m = dp("out_wT_m", [LM, D, D], F32, isOutput=False)
    in_wT_s = dp("in_wT_s", [LS, D, 3 * D], F32, isOutput=False)
    out_wT_s = dp("out_wT_s", [LS, D, D], F32, isOutput=False)
    wedt = BF16 if EXPERT_BF16 else F32
    w1T = dp("w1T", [LM, E, D, DFF], wedt, isOutput=False)
    w2T = dp("w2T", [LM, E, DFF, D], wedt, isOutput=False)
    identb = dp("identb", [P, P], BF16, isOutput=False)
    fc_wT = dp("fc_wT", [LS, D, DFF], F32, isOutput=False)
    proj_wT = dp("proj_wT", [LS, DFF, D], F32, isOutput=False)
    mp0 = dp("mp0", [P, PK_MOE_COLS], F32, isOutput=False)
    mp1 = dp("mp1", [P, PK_MOE_COLS], F32, isOutput=False)
    sp0 = dp("sp0", [P, PK_STD_COLS], F32, isOutput=False)
    sp1 = dp("sp1", [P, PK_STD_COLS], F32, isOutput=False)
    yT = dp("yT", [D, T], F32, isOutput=True)
    logits_out = dp("logits_out", [TP, E], F32, isOutput=True)

    xydt = BF16 if EXPERT_BF16 else F32
    Xbuf = nc.dram_tensor("Xbuf", [XROWS, D], xydt)
    Ybuf = nc.dram_tensor("Ybuf", [XROWS, D], xydt)

    with tile.TileContext(nc) as tc:
        with ExitStack() as ctx:
            build_body(nc, tc, ctx, locals())
    nc.finalize()
    return nc


def build_body(nc, tc, ctx, t_):
    xT, ident, lstrict, allon = t_["xT"], t_["ident"], t_["lstrict"], t_["allon"]
    ec8, capmax, validc, trashc = t_["ec8"], t_["capmax"], t_["validc"], t_["trashc"]
    gate_wT = t_["gate_wT"]
    in_wT_m, out_wT_m = t_["in_wT_m"], t_["out_wT_m"]
    in_wT_s, out_wT_s = t_["in_wT_s"], t_["out_wT_s"]
    w1T, w2T, fc_wT, proj_wT = t_["w1T"], t_["w2T"], t_["fc_wT"], t_["proj_wT"]
    packs = [t_["mp0"], t_["mp1"], t_["sp0"], t_["sp1"]]
    yT, logits_out = t_["yT"], t_["logits_out"]
    Xbuf, Ybuf = t_["Xbuf"], t_["Ybuf"]

    consts = ctx.enter_context(tc.tile_pool(name="consts", bufs=1))
    resid = ctx.enter_context(tc.tile_pool(name="resid", bufs=1))
    route = ctx.enter_context(tc.tile_pool(name="route", bufs=1))
    parms = ctx.enter_context(tc.tile_pool(name="parms", bufs=1))

    id_f = consts.tile([P, P], F32, name="id_f", tag="id_f")
    nc.sync.dma_start(out=id_f, in_=ident[:, :])
    id_b = consts.tile([P, P], BF16, name="id_b", tag="id_b")
    nc.sync.dma_start(out=id_b, in_=t_["identb"][:, :])
    ls_f = consts.tile([P, P], F32, name="ls_f", tag="ls_f")
    nc.sync.dma_start(out=ls_f, in_=lstrict[:, :])
    on_f = consts.tile([P, P], F32, name="on_f", tag="on_f")
    nc.sync.dma_start(out=on_f, in_=allon[:, :])
    on_r = consts.tile([P, P], F32R, name="on_r", tag="on_r")
    nc.sync.dma_start(out=on_r, in_=allon[:, :].bitcast(F32R))
    ec_sb = consts.tile([P, E], F32, name="ec_sb", tag="ec_sb")
    nc.sync.dma_start(out=ec_sb, in_=ec8[:, :])
    cm_sb = consts.tile([P, E], F32, name="cm_sb", tag="cm_sb")
    nc.sync.dma_start(out=cm_sb, in_=capmax[:, :])
    va_sb = consts.tile([P, NT], F32, name="va_sb", tag="va_sb")
    nc.sync.dma_start(out=va_sb, in_=validc[:, :])
    tr_sb = consts.tile([P, NT], F32, name="tr_sb", tag="tr_sb")
    nc.sync.dma_start(out=tr_sb, in_=trashc[:, :])
    eps_r = consts.tile([1, 1], F32, name="eps_r", tag="eps_r")
    nc.vector.memset(eps_r, EPS)
    gw_sb = consts.tile([P, DT, E], F32, name="gw_sb", tag="gw_sb")
    nc.sync.dma_start(out=gw_sb, in_=gate_wT.rearrange("(a p) e -> p a e", p=P))

    # residual stream, feature-major [6][128, TP]
    x_t = []
    for dt in range(DT):
        xx = resid.tile([P, TP], F32, name=f"x{dt}", tag=f"x{dt}")
        nc.sync.dma_start(out=xx, in_=xT[dt * P:(dt + 1) * P, :])
        x_t.append(xx)

    # routing state (computed in layer 0, reused in layer 1)
    dr0_i = [route.tile([P, 1], I32, name=f"dr0_{i}", tag=f"dr0_{i}") for i in range(NT)]
    dr1_i = [route.tile([P, 1], I32, name=f"dr1_{i}", tag=f"dr1_{i}") for i in range(NT)]
    cw0_t = [route.tile([P, 1], F32, name=f"cw0_{i}", tag=f"cw0_{i}") for i in range(NT)]
    cw1_t = [route.tile([P, 1], F32, name=f"cw1_{i}", tag=f"cw1_{i}") for i in range(NT)]

    layers = [("moe", 0), ("moe", 1), ("std", 0), ("std", 1)]
    for L, (kind, li) in enumerate(layers):
        f32_mode = (L == 0)
        adt = F32 if f32_mode else F32R  # activation dtype for matmul operands
        pk_cols = PK_MOE_COLS if kind == "moe" else PK_STD_COLS
        pk = parms.tile([P, pk_cols], F32, name=f"pk{L}", tag="pk", bufs=1)
        nc.sync.dma_start(out=pk, in_=packs[L][:, :])
        in_wT = in_wT_m if kind == "moe" else in_wT_s
        out_wT = out_wT_m if kind == "moe" else out_wT_s

        attention(nc, tc, x_t, pk, in_wT, out_wT, li, adt, f32_mode, on_f, on_r, eps_r)

        if kind == "moe":
            moe_ffn(nc, tc, x_t, pk, li, adt, f32_mode, L,
                    w1T, w2T, Xbuf, Ybuf, logits_out, gw_sb,
                    id_f, id_b, ls_f, on_f, on_r, ec_sb, cm_sb, va_sb, tr_sb,
                    dr0_i, dr1_i, cw0_t, cw1_t, eps_r)
        else:
            std_ffn(nc, tc, x_t, pk, li, adt, fc_wT, proj_wT, on_f, on_r, eps_r)

    for dt in range(DT):
        nc.sync.dma_start(out=yT[dt * P:(dt + 1) * P, :], in_=x_t[dt][:, 0:T])


def layer_norm_chunk(nc, x_t, cols, W, s_col, b_col, pk, out_tiles, out_cols,
                     adt, f32_mode, on_f, on_r, sq_pool, row_pool, psp, tmp_pool,
                     eps_r=None):
    """LN over features (partitions) for a column chunk. out_tiles[dt][:, out_cols] = LN.

    Stats always run in fp32 (x tiles are fp32-produced; f32r matmul inputs must
    be produced-as-f32r). Broadcast matmuls use f32r row copies when allowed.
    """
    p_s = psp.tile([1, W], F32, name="p_s", tag="acc", bufs=2)
    p_q = psp.tile([1, W], F32, name="p_q", tag="acc", bufs=2)
    for dt in range(DT):
        sq = sq_pool.tile([P, W], F32, name="sq", tag="sq")
        nc.scalar.activation(out=sq, in_=x_t[dt][:, cols], func=AF.Square)
        nc.tensor.matmul(p_s, on_f[:, 0:1], x_t[dt][:, cols],
                         start=(dt == 0), stop=(dt == DT - 1))
        nc.tensor.matmul(p_q, on_f[:, 0:1], sq, start=(dt == 0), stop=(dt == DT - 1))
    mu = row_pool.tile([1, W], F32, name="mu", tag="mu")
    nc.scalar.mul(out=mu, in_=p_s, mul=1.0 / D)
    msq = row_pool.tile([1, W], F32, name="msq", tag="msq")
    nc.scalar.activation(out=msq, in_=mu, func=AF.Square)
    var = row_pool.tile([1, W], F32, name="var", tag="var")
    nc.scalar.mul(out=var, in_=p_q, mul=1.0 / D)
    nc.vector.tensor_sub(out=var, in0=var, in1=msq)
    sd = row_pool.tile([1, W], F32, name="sd", tag="sd")
    nc.scalar.activation(out=sd, in_=var, func=AF.Sqrt, bias=eps_r[0:1, 0:1])
    rs = row_pool.tile([1, W], F32, name="rs", tag="rs")
    nc.vector.reciprocal(out=rs, in_=sd)
    if f32_mode:
        on_x, mu_b, rs_b = on_f, mu, rs
    else:
        on_x = on_r
        mu_b = row_pool.tile([1, W], F32R, name="mu_r", tag="mu_r")
        nc.scalar.copy(out=mu_b, in_=mu)
        rs_b = row_pool.tile([1, W], F32R, name="rs_r", tag="rs_r")
        nc.scalar.copy(out=rs_b, in_=rs)
    p_mu = psp.tile([P, W], F32, name="p_mu", tag="acc", bufs=2)
    nc.tensor.matmul(p_mu, on_x[0:1, :], mu_b, start=True, stop=True)
    p_rs = psp.tile([P, W], F32, name="p_rs", tag="acc", bufs=2)
    nc.tensor.matmul(p_rs, on_x[0:1, :], rs_b, start=True, stop=True)
    for dt in range(DT):
        tmp = tmp_pool.tile([P, W], F32, name="lntmp", tag="lntmp")
        nc.vector.tensor_sub(out=tmp, in0=x_t[dt][:, cols], in1=p_mu)
        nc.vector.tensor_mul(out=tmp, in0=tmp, in1=p_rs)
        nc.vector.tensor_scalar(
            out=out_tiles[dt][:, out_cols], in0=tmp,
            scalar1=pk[:, s_col + dt:s_col + dt + 1],
            scalar2=pk[:, b_col + dt:b_col + dt + 1],
            op0=ALU.mult, op1=ALU.add)


def attention(nc, tc, x_t, pk, in_wT, out_wT, li, adt, f32_mode, on_f, on_r, eps_r):
    with ExitStack() as ctx:
        wq_p = ctx.enter_context(tc.tile_pool(name="wqkv", bufs=1))
        wo_p = ctx.enter_context(tc.tile_pool(name="wout", bufs=2))
        ln_p = ctx.enter_context(tc.tile_pool(name="ln1", bufs=2))
        qk_p = ctx.enter_context(tc.tile_pool(name="qk", bufs=1))
        v_p = ctx.enter_context(tc.tile_pool(name="vtok", bufs=6))
        o_p = ctx.enter_context(tc.tile_pool(name="oT", bufs=1))
        e_p = ctx.enter_context(tc.tile_pool(name="expt", bufs=5))
        sm_p = ctx.enter_context(tc.tile_pool(name="smrow", bufs=4))
        sq_p = ctx.enter_context(tc.tile_pool(name="sqt", bufs=1))
        row_p = ctx.enter_context(tc.tile_pool(name="lnrow", bufs=1))
        tmp_p = ctx.enter_context(tc.tile_pool(name="lntmp", bufs=2))
        psp = ctx.enter_context(tc.tile_pool(name="ps_att", bufs=1, space="PSUM"))

        wdt = F32 if f32_mode else F32R
        # qkv weight strips resident for the layer: [6][128, 2304]
        wqkv = []
        for dt in range(DT):
            w = wq_p.tile([P, 3 * D], wdt, name=f"wqkv{dt}", tag=f"wqkv{dt}")
            src = in_wT[li, dt * P:(dt + 1) * P, :]
            nc.sync.dma_start(out=w, in_=src if f32_mode else src.bitcast(F32R))
            wqkv.append(w)

        for c in range(CH):
            c0 = c * CW
            cols = slice(c0, c0 + CW)
            ln1 = [ln_p.tile([P, CW], adt, name=f"ln1_{dt}", tag=f"ln1_{dt}") for dt in range(DT)]
            layer_norm_chunk(nc, x_t, cols, CW, PK_LN1S, PK_LN1B, pk, ln1,
                             slice(0, CW), adt, f32_mode, on_f, on_r,
                             sq_p, row_p, psp, tmp_p, eps_r)
            # q,k feature-major: [12][128, 453] (padded width for rhs windows)
            QW = 394
            qk = [qk_p.tile([P, QW], adt, name=f"qk{ot}", tag=f"qk{ot}") for ot in range(12)]
            for ot in range(12):
                ps = psp.tile([P, CW], F32, name="ps_qk", tag="acc", bufs=2)
                for dt in range(DT):
                    nc.tensor.matmul(ps, wqkv[dt][:, ot * P:(ot + 1) * P], ln1[dt],
                                     start=(dt == 0), stop=(dt == DT - 1))
                nc.vector.tensor_scalar(
                    out=qk[ot][:, 0:CW], in0=ps,
                    scalar1=pk[:, PK_INB + ot:PK_INB + ot + 1], scalar2=None, op0=ALU.add)
            # v token-major per b-block: [2 tiles per b][*, 768]
            v_b = {}
            for b2 in range(2):
                for kt in range(2):
                    kh = P if kt == 0 else S - P  # 128 / 69
                    vt = v_p.tile([P, D], adt, name=f"v_{b2}_{kt}", tag="v")
                    tcol0 = b2 * S + kt * P
                    for nch in range(2):
                        ncols = slice(nch * 384, (nch + 1) * 384)
                        ps = psp.tile([P, 384], F32, name="ps_v", tag="acc", bufs=2)
                        for dt in range(DT):
                            nc.tensor.matmul(
                                ps[0:kh, :], ln1[dt][:, tcol0:tcol0 + kh],
                                wqkv[dt][:, 1536 + nch * 384:1536 + (nch + 1) * 384],
                                start=(dt == 0), stop=(dt == DT - 1))
                        nc.vector.tensor_tensor(
                            out=vt[0:kh, ncols], in0=ps[0:kh, :],
                            in1=pk[0:kh, PK_VB + nch * 384:PK_VB + (nch + 1) * 384],
                            op=ALU.add)
                    v_b[(b2, kt)] = vt
            # attention core per (b2, h)
            oT = [o_p.tile([P, CW], adt, name=f"oT{ot}", tag=f"oT{ot}") for ot in range(DT)]
            for b2 in range(2):
                q0 = b2 * S
                QS = 0 if b2 == 0 else CW - 256  # window start; q cols at QS.. fit in tile
                QO = q0 - QS  # offset of real q0 within the 256-window
                for h in range(H):
                    otq, po = h // 2, (h % 2) * HD
                    exps = []
                    for kt in range(2):
                        kh = P if kt == 0 else S - P
                        pss = psp.tile([P, 256], F32, name="ps_sc", tag="sc", bufs=4)
                        nc.tensor.matmul(
                            pss[0:kh, :],
                            qk[6 + otq][po:po + HD, q0 + kt * P:q0 + kt * P + kh],
                            qk[otq][po:po + HD, QS:QS + 256],
                            start=True, stop=True)
                        ex = e_p.tile([P, 256], adt, name="ex", tag="ex")
                        nc.scalar.activation(out=ex[0:kh, :], in_=pss[0:kh, :],
                                             func=AF.Exp, scale=0.125)
                        exps.append((ex, kh))
                    on_x = on_f if f32_mode else on_r
                    p_den = psp.tile([1, 256], F32, name="p_den", tag="acc", bufs=2)
                    for kt, (ex, kh) in enumerate(exps):
                        nc.tensor.matmul(p_den, on_x[0:kh, 0:1], ex[0:kh, :],
                                         start=(kt == 0), stop=(kt == 1))
                    rec = sm_p.tile([1, 256], adt, name="rec", tag="rec")
                    with nc.allow_low_precision("softmax recip in f32r"):
                        nc.vector.reciprocal(out=rec, in_=p_den)
                    p_bc = psp.tile([HD, 256], F32, name="p_bc", tag="ob", bufs=2)
                    nc.tensor.matmul(p_bc, on_x[0:1, 0:HD], rec, start=True, stop=True)
                    bc = sm_p.tile([HD, 256], F32, name="bc", tag="bc")
                    nc.scalar.copy(out=bc, in_=p_bc)
                    p_o = psp.tile([HD, 256], F32, name="p_o", tag="ob", bufs=2)
                    for kt, (ex, kh) in enumerate(exps):
                        nc.tensor.matmul(p_o, v_b[(b2, kt)][0:kh, h * HD:(h + 1) * HD],
                                         ex[0:kh, :], start=(kt == 0), stop=(kt == 1))
                    nc.vector.tensor_tensor(
                        out=oT[otq][po:po + HD, q0:q0 + S],
                        in0=p_o[:, QO:QO + S], in1=bc[:, QO:QO + S], op=ALU.mult)
            # out-proj + residual (in place): stream wout strips once per chunk,
            # 6 accumulators borrowed from the (now idle) sc/ob psum tags
            psop = []
            for dt in range(DT):
                tg = "sc" if dt < 4 else "ob"
                bf = 4 if dt < 4 else 2
                psop.append(psp.tile([P, CW], F32, name=f"psop{dt}", tag=tg, bufs=bf))
            for od in range(DT):
                w = wo_p.tile([P, D], wdt, name="wout", tag="wout")
                src = out_wT[li, od * P:(od + 1) * P, :]
                nc.sync.dma_start(out=w, in_=src if f32_mode else src.bitcast(F32R))
                for dt in range(DT):
                    nc.tensor.matmul(psop[dt], w[:, dt * P:(dt + 1) * P], oT[od],
                                     start=(od == 0), stop=(od == DT - 1))
            for dt in range(DT):
                tmp = tmp_p.tile([P, CW], F32, name="aotmp", tag="lntmp")
                nc.vector.tensor_scalar(
                    out=tmp, in0=psop[dt],
                    scalar1=pk[:, PK_OUTB + dt:PK_OUTB + dt + 1], scalar2=None, op0=ALU.add)
                nc.vector.tensor_add(out=x_t[dt][:, cols], in0=x_t[dt][:, cols], in1=tmp)


def moe_ffn(nc, tc, x_t, pk, li, adt, f32_mode, L, w1T, w2T, Xbuf, Ybuf,
            logits_out, gw_sb, id_f, id_b, ls_f, on_f, on_r, ec_sb, cm_sb, va_sb, tr_sb,
            dr0_i, dr1_i, cw0_t, cw1_t, eps_r):
    # --- LN2 into resident hsT (fp32; dispatch + logits consume it) ---
    with ExitStack() as ctx:
        hs_p = ctx.enter_context(tc.tile_pool(name="hsT", bufs=1))
        sq_p = ctx.enter_context(tc.tile_pool(name="sqt2", bufs=2))
        row_p = ctx.enter_context(tc.tile_pool(name="lnrow2", bufs=1))
        tmp_p = ctx.enter_context(tc.tile_pool(name="lntmp2", bufs=2))
        psp = ctx.enter_context(tc.tile_pool(name="ps_moe", bufs=1, space="PSUM"))

        hsT = [hs_p.tile([P, TP], F32, name=f"hsT{dt}", tag=f"hsT{dt}") for dt in range(DT)]
        for c in range(CH):
            cols = slice(c * CW, c * CW + CW)
            layer_norm_chunk(nc, x_t, cols, CW, PK_LN2S, PK_LN2B, pk, hsT, cols,
                             F32, True, on_f, on_r, sq_p, row_p, psp, tmp_p, eps_r)
        for dt in range(DT):
            nc.vector.memset(hsT[dt][:, T:TP], 0.0)

        if L == 0:
            # --- routing: logits, top-2, combine weights, slot positions ---
            rt_p = ctx.enter_context(tc.tile_pool(name="rt", bufs=3))
            sel_p = ctx.enter_context(tc.tile_pool(name="selp", bufs=1))
            sel_t, m1_t, m2_t = [], [], []
            for i in range(NT):
                tc0 = i * P
                ps_lg = psp.tile([P, E], F32, name="ps_lg", tag="acc", bufs=2)
                for dt in range(DT):
                    nc.tensor.matmul(ps_lg, hsT[dt][:, tc0:tc0 + P], gw_sb[:, dt, :],
                                     start=(dt == 0), stop=(dt == DT - 1))
                lg = rt_p.tile([P, E], F32, name="lg", tag="lg")
                nc.vector.tensor_copy(out=lg, in_=ps_lg)
                nc.sync.dma_start(out=logits_out[tc0:tc0 + P, :], in_=lg)
                srt = rt_p.tile([P, E], F32, name="srt", tag="srt")
                nc.vector.max(out=srt, in_=lg)
                em2 = rt_p.tile([P, 1], F32, name="em2", tag="em2")
                nc.vector.tensor_sub(out=em2, in0=srt[:, 1:2], in1=srt[:, 0:1])
                nc.scalar.activation(out=em2, in_=em2, func=AF.Exp)
                den = rt_p.tile([P, 1], F32, name="den", tag="den")
                nc.scalar.activation(out=den, in_=em2, func=AF.Identity, bias=on_f[:, 0:1])
                nc.vector.reciprocal(out=cw0_t[i], in_=den)
                nc.vector.tensor_mul(out=cw1_t[i], in0=em2, in1=cw0_t[i])
                m1 = sel_p.tile([P, E], F32, name=f"m1_{i}", tag=f"m1_{i}")
                nc.vector.tensor_scalar(out=m1, in0=lg, scalar1=srt[:, 0:1],
                                        scalar2=None, op0=ALU.is_ge)
                sel = sel_p.tile([P, E], F32, name=f"sel_{i}", tag=f"sel_{i}")
                nc.vector.tensor_scalar(out=sel, in0=lg, scalar1=srt[:, 1:2],
                                        scalar2=None, op0=ALU.is_ge)
                m2 = sel_p.tile([P, E], F32, name=f"m2_{i}", tag=f"m2_{i}")
                nc.vector.tensor_sub(out=m2, in0=sel, in1=m1)
                nc.vector.tensor_scalar_mul(out=sel, in0=sel, scalar1=va_sb[:, i:i + 1])
                sel_t.append(sel); m1_t.append(m1); m2_t.append(m2)
            for i in range(NT):
                ps_pos = psp.tile([P, E], F32, name="ps_pos", tag="acc", bufs=2)
                nc.tensor.matmul(ps_pos, ls_f, sel_t[i], start=True, stop=(i == 0))
                for j in range(i):
                    nc.tensor.matmul(ps_pos, on_f, sel_t[j], start=False, stop=(j == i - 1))
                drf = rt_p.tile([P, E], F32, name="drf", tag="drf")
                nc.vector.tensor_tensor(out=drf, in0=ps_pos, in1=ec_sb, op=ALU.add)
                nc.vector.tensor_tensor(out=drf, in0=drf, in1=cm_sb, op=ALU.min)
                for msk, dri in ((m1_t[i], dr0_i[i]), (m2_t[i], dr1_i[i])):
                    t0 = rt_p.tile([P, E], F32, name="selr", tag="selr")
                    nc.vector.tensor_mul(out=t0, in0=drf, in1=msk)
                    r0 = rt_p.tile([P, 1], F32, name="r0", tag="r0")
                    nc.vector.reduce_sum(out=r0, in_=t0, axis=mybir.AxisListType.X)
                    nc.vector.tensor_scalar(out=r0, in0=r0, scalar1=va_sb[:, i:i + 1],
                                            scalar2=None, op0=ALU.mult)
                    nc.vector.tensor_scalar(out=r0, in0=r0, scalar1=tr_sb[:, i:i + 1],
                                            scalar2=None, op0=ALU.add)
                    nc.vector.tensor_copy(out=dri, in_=r0)

        # --- dispatch: transpose hs to token-major tiles, scatter to Xbuf ---
        tok_p = ctx.enter_context(tc.tile_pool(name="tok", bufs=4))
        for i in range(NT):
            tc0 = i * P
            ht = tok_p.tile([P, D], BF16 if EXPERT_BF16 else F32, name="hstok", tag="hstok")
            for dt in range(DT):
                pt = psp.tile([P, P], F32, name="pt", tag="tr", bufs=4)
                nc.tensor.transpose(out=pt, in_=hsT[dt][:, tc0:tc0 + P], identity=id_f)
                nc.vector.tensor_copy(out=ht[:, dt * P:(dt + 1) * P], in_=pt)
            for dri in (dr0_i[i], dr1_i[i]):
                nc.gpsimd.indirect_dma_start(
                    out=Xbuf[:, :],
                    out_offset=bass.IndirectOffsetOnAxis(ap=dri[:, 0:1], axis=0),
                    in_=ht, in_offset=None)

    # --- experts ---
    with ExitStack() as ctx:
        xe_p = ctx.enter_context(tc.tile_pool(name="xe", bufs=4))
        xet_p = ctx.enter_context(tc.tile_pool(name="xet", bufs=1))
        y1_p = ctx.enter_context(tc.tile_pool(name="y1", bufs=1))
        y2_p = ctx.enter_context(tc.tile_pool(name="y2", bufs=1))
        w1_p = ctx.enter_context(tc.tile_pool(name="w1p", bufs=18))
        w2_p = ctx.enter_context(tc.tile_pool(name="w2p", bufs=6))
        sg_p = ctx.enter_context(tc.tile_pool(name="sgp", bufs=3))
        ye_p = ctx.enter_context(tc.tile_pool(name="yep", bufs=3))
        ps_e = ctx.enter_context(tc.tile_pool(name="ps_exp", bufs=1, space="PSUM"))

        for e in range(E):
            edt = BF16 if EXPERT_BF16 else F32R
            xid = id_b if EXPERT_BF16 else id_f
            xet = [xet_p.tile([P, CAP], edt, name=f"xet{dt}", tag=f"xet{dt}")
                   for dt in range(DT)]
            for i in range(CAP // P):
                xe = xe_p.tile([P, D], BF16 if EXPERT_BF16 else F32, name="xe", tag="xe")
                r0 = e * CAP + i * P
                nc.sync.dma_start(out=xe, in_=Xbuf[r0:r0 + P, :])
                for dt in range(DT):
                    pt = ps_e.tile([P, P], BF16 if EXPERT_BF16 else F32, name="pt1",
                                   tag="pmm", bufs=2)
                    nc.tensor.transpose(out=pt, in_=xe[:, dt * P:(dt + 1) * P], identity=xid)
                    nc.vector.tensor_copy(out=xet[dt][:, i * P:(i + 1) * P], in_=pt)
            y1t = [y1_p.tile([P, CAP], edt, name=f"y1t{ft}", tag=f"y1t{ft}")
                   for ft in range(FT)]
            for fg in range(6):
                w1s = []
                for dt in range(DT):
                    w = w1_p.tile([P, 4 * P], edt, name="w1s", tag="w1s")
                    src = w1T[li, e, dt * P:(dt + 1) * P, fg * 4 * P:(fg + 1) * 4 * P]
                    nc.sync.dma_start(out=w, in_=src if EXPERT_BF16 else src.bitcast(F32R))
                    w1s.append(w)
                for f4 in range(4):
                    ft = fg * 4 + f4
                    ps = ps_e.tile([P, CAP], F32, name="ps_y1", tag="pmm", bufs=2)
                    for dt in range(DT):
                        nc.tensor.matmul(ps, w1s[dt][:, f4 * P:(f4 + 1) * P], xet[dt],
                                         start=(dt == 0), stop=(dt == DT - 1))
                    b1 = pk[:, PK_MOE_B1 + e * FT + ft:PK_MOE_B1 + e * FT + ft + 1]
                    b1s = pk[:, PK_MOE_B1S + e * FT + ft:PK_MOE_B1S + e * FT + ft + 1]
                    sg = sg_p.tile([P, CAP], F32, name="sg", tag="sg")
                    nc.scalar.activation(out=sg, in_=ps, func=AF.Sigmoid,
                                         bias=b1s, scale=1.702)
                    t1 = sg_p.tile([P, CAP], F32, name="t1", tag="t1")
                    nc.vector.tensor_scalar(out=t1, in0=ps, scalar1=b1,
                                            scalar2=None, op0=ALU.add)
                    nc.vector.tensor_mul(out=y1t[ft], in0=t1, in1=sg)
            y2t = [y2_p.tile([P, CAP], F32, name=f"y2t{dt}", tag=f"y2t{dt}")
                   for dt in range(DT)]
            ps6 = [ps_e.tile([P, CAP], F32, name=f"ps6_{dt}", tag=f"ps6_{dt}", bufs=1)
                   for dt in range(DT)]
            for ft in range(FT):
                w = w2_p.tile([P, D], edt, name="w2s", tag="w2s")
                src2 = w2T[li, e, ft * P:(ft + 1) * P, :]
                nc.sync.dma_start(out=w, in_=src2 if EXPERT_BF16 else src2.bitcast(F32R))
                for dt in range(DT):
                    nc.tensor.matmul(ps6[dt], w[:, dt * P:(dt + 1) * P], y1t[ft],
                                     start=(ft == 0), stop=(ft == FT - 1))
            for dt in range(DT):
                b2 = pk[:, PK_MOE_B2 + e * DT + dt:PK_MOE_B2 + e * DT + dt + 1]
                nc.vector.tensor_scalar(out=y2t[dt], in0=ps6[dt], scalar1=b2,
                                        scalar2=None, op0=ALU.add)
            for i in range(CAP // P):
                ye = ye_p.tile([P, D], BF16 if EXPERT_BF16 else F32, name="ye", tag="ye")
                for dt in range(DT):
                    pt = ps_e.tile([P, P], F32, name="pt2", tag="pmm", bufs=2)
                    nc.tensor.transpose(out=pt, in_=y2t[dt][:, i * P:(i + 1) * P],
                                        identity=id_f)
                    nc.vector.tensor_copy(out=ye[:, dt * P:(dt + 1) * P], in_=pt)
                r0 = e * CAP + i * P
                nc.sync.dma_start(out=Ybuf[r0:r0 + P, :], in_=ye)

    # --- combine: gather per token tile, weighted sum, residual add ---
    with ExitStack() as ctx:
        g_p = ctx.enter_context(tc.tile_pool(name="gat", bufs=6))
        ps_t = ctx.enter_context(tc.tile_pool(name="ps_t2", bufs=1, space="PSUM"))
        for i in range(NT):
            tc0 = i * P
            gdt = BF16 if EXPERT_BF16 else F32
            y0 = g_p.tile([P, D], gdt, name="y0g", tag="y0g")
            nc.gpsimd.indirect_dma_start(
                out=y0, out_offset=None, in_=Ybuf[:, :],
                in_offset=bass.IndirectOffsetOnAxis(ap=dr0_i[i][:, 0:1], axis=0))
            y1g = g_p.tile([P, D], gdt, name="y1g", tag="y1g")
            nc.gpsimd.indirect_dma_start(
                out=y1g, out_offset=None, in_=Ybuf[:, :],
                in_offset=bass.IndirectOffsetOnAxis(ap=dr1_i[i][:, 0:1], axis=0))
            cmb = g_p.tile([P, D], F32, name="cmb", tag="cmb")
            nc.vector.tensor_scalar_mul(out=cmb, in0=y0, scalar1=cw0_t[i])
            t2 = g_p.tile([P, D], F32, name="t2g", tag="t2g")
            nc.vector.tensor_scalar_mul(out=t2, in0=y1g, scalar1=cw1_t[i])
            nc.vector.tensor_add(out=cmb, in0=cmb, in1=t2)
            for dt in range(DT):
                pt = ps_t.tile([P, P], F32, name="pt3", tag="tr2", bufs=4)
                nc.tensor.transpose(out=pt, in_=cmb[:, dt * P:(dt + 1) * P], identity=id_f)
                nc.vector.tensor_add(out=x_t[dt][:, tc0:tc0 + P],
                                     in0=x_t[dt][:, tc0:tc0 + P], in1=pt)


def std_ffn(nc, tc, x_t, pk, li, adt, fc_wT, proj_wT, on_f, on_r, eps_r):
    with ExitStack() as ctx:
        hs_p = ctx.enter_context(tc.tile_pool(name="hs_s", bufs=1))
        h1_p = ctx.enter_context(tc.tile_pool(name="h1", bufs=1))
        wf_p = ctx.enter_context(tc.tile_pool(name="wfc", bufs=1))
        wp_p = ctx.enter_context(tc.tile_pool(name="wproj", bufs=3))
        sg_p = ctx.enter_context(tc.tile_pool(name="sg_s", bufs=2))
        sq_p = ctx.enter_context(tc.tile_pool(name="sqt3", bufs=2))
        row_p = ctx.enter_context(tc.tile_pool(name="lnrow3", bufs=1))
        tmp_p = ctx.enter_context(tc.tile_pool(name="lntmp3", bufs=2))
        ps_st = ctx.enter_context(tc.tile_pool(name="ps_std", bufs=1, space="PSUM"))

        for c in range(CH):
            cols = slice(c * CW, c * CW + CW)
            hs = [hs_p.tile([P, CW], F32R, name=f"hs_{dt}", tag=f"hs_{dt}") for dt in range(DT)]
            layer_norm_chunk(nc, x_t, cols, CW, PK_LN2S, PK_LN2B, pk, hs,
                             slice(0, CW), F32R, False, on_f, on_r,
                             sq_p, row_p, ps_st, tmp_p, eps_r)
            h1 = [h1_p.tile([P, CW], F32R, name=f"h1_{ft}", tag=f"h1_{ft}") for ft in range(FT)]
            for half in range(2):
                wfc = []
                for dt in range(DT):
                    w = wf_p.tile([P, DFF // 2], F32R, name=f"wfc{dt}", tag=f"wfc{dt}")
                    nc.sync.dma_start(
                        out=w, in_=fc_wT[li, dt * P:(dt + 1) * P,
                                         half * (DFF // 2):(half + 1) * (DFF // 2)].bitcast(F32R))
                    wfc.append(w)
                for ft in range(half * 12, half * 12 + 12):
                    fo = ft - half * 12
                    ps = ps_st.tile([P, CW], F32, name="ps_fc", tag="acc", bufs=2)
                    for dt in range(DT):
                        nc.tensor.matmul(ps, wfc[dt][:, fo * P:(fo + 1) * P], hs[dt],
                                         start=(dt == 0), stop=(dt == DT - 1))
                    fcb = pk[:, PK_FCB + ft:PK_FCB + ft + 1]
                    fcbs = pk[:, PK_FCBS + ft:PK_FCBS + ft + 1]
                    sg = sg_p.tile([P, CW], F32, name="sg_f", tag="sg_f")
                    nc.scalar.activation(out=sg, in_=ps, func=AF.Sigmoid, bias=fcbs, scale=1.702)
                    t1 = sg_p.tile([P, CW], F32, name="t1_f", tag="t1_f")
                    nc.vector.tensor_scalar(out=t1, in0=ps, scalar1=fcb, scalar2=None, op0=ALU.add)
                    nc.vector.tensor_mul(out=h1[ft], in0=t1, in1=sg)
            ps6 = [ps_st.tile([P, CW], F32, name=f"ps6s_{dt}", tag=f"ps6s_{dt}", bufs=1)
                   for dt in range(DT)]
            for ft in range(FT):
                w = wp_p.tile([P, D], F32R, name="wpj", tag="wpj")
                nc.sync.dma_start(out=w, in_=proj_wT[li, ft * P:(ft + 1) * P, :].bitcast(F32R))
                for dt in range(DT):
                    nc.tensor.matmul(ps6[dt], w[:, dt * P:(dt + 1) * P], h1[ft],
                                     start=(ft == 0), stop=(ft == FT - 1))
            for dt in range(DT):
                tmp = tmp_p.tile([P, CW], F32, name="pjtmp", tag="lntmp3")
                nc.vector.tensor_scalar(
                    out=tmp, in0=ps6[dt],
                    scalar1=pk[:, PK_PROJB + dt:PK_PROJB + dt + 1], scalar2=None, op0=ALU.add)
                nc.vector.tensor_add(out=x_t[dt][:, cols], in0=x_t[dt][:, cols], in1=tmp)


# ---------------- host side ----------------

_CACHE = {}


def _pack_vec(pack, col, v):
    n = v.shape[0] // P
    pack[:, col:col + n] = v.reshape(n, P).T


def _build_host_constants():
    ident = np.eye(P, dtype=np.float32)
    lstrict = np.triu(np.ones((P, P), np.float32), 1)
    allon = np.ones((P, P), np.float32)
    ec8 = np.tile((np.arange(E) * CAP).astype(np.float32), (P, 1))
    capmax = np.tile(((np.arange(E) + 1) * CAP - 1).astype(np.float32), (P, 1))
    validc = np.zeros((P, NT), np.float32)
    trashc = np.zeros((P, NT), np.float32)
    for i in range(NT):
        for p in range(P):
            if i * P + p < T:
                validc[p, i] = 1.0
            else:
                trashc[p, i] = float(TRASH)
    return ident, lstrict, allon, ec8, capmax, validc, trashc


def _make_packs(inputs):
    packs = []
    for kind, li in [("moe", 0), ("moe", 1), ("std", 0), ("std", 1)]:
        pfx = "moe" if kind == "moe" else "std"
        cols = PK_MOE_COLS if kind == "moe" else PK_STD_COLS
        pack = np.zeros((P, cols), np.float32)
        _pack_vec(pack, PK_LN1S, inputs[f"{pfx}_ln1_s"][li])
        _pack_vec(pack, PK_LN1B, inputs[f"{pfx}_ln1_b"][li])
        _pack_vec(pack, PK_INB, inputs[f"{pfx}_in_b"][li])
        _pack_vec(pack, PK_OUTB, inputs[f"{pfx}_out_b"][li])
        _pack_vec(pack, PK_LN2S, inputs[f"{pfx}_ln2_s"][li])
        _pack_vec(pack, PK_LN2B, inputs[f"{pfx}_ln2_b"][li])
        pack[:, PK_VB:PK_VB + D] = np.tile(inputs[f"{pfx}_in_b"][li][2 * D:], (P, 1))
        if kind == "moe":
            for e in range(E):
                _pack_vec(pack, PK_MOE_B1 + e * FT, inputs["moe_b1"][li, e])
                _pack_vec(pack, PK_MOE_B1S + e * FT,
                          (1.702 * inputs["moe_b1"][li, e]).astype(np.float32))
                _pack_vec(pack, PK_MOE_B2 + e * DT, inputs["moe_b2"][li, e])
        else:
            _pack_vec(pack, PK_FCB, inputs["std_fc_b"][li])
            _pack_vec(pack, PK_FCBS, (1.702 * inputs["std_fc_b"][li]).astype(np.float32))
            _pack_vec(pack, PK_PROJB, inputs["std_proj_b"][li])
        packs.append(pack)
    return packs


def _prep_core_inputs(inputs, core):
    """Everything except xT is shared across cores; build shared dict once."""
    c = np.ascontiguousarray
    tr = lambda a: c(np.swapaxes(a, -1, -2))
    ident, lstrict, allon, ec8, capmax, validc, trashc = _build_host_constants()
    mp0, mp1, sp0, sp1 = _make_packs(inputs)
    wcast = (lambda a: a.astype(ml_dtypes.bfloat16)) if EXPERT_BF16 else (lambda a: a)
    shared = dict(
        ident=ident, lstrict=lstrict, allon=allon, ec8=ec8, capmax=capmax,
        validc=validc, trashc=trashc, identb=ident.astype(ml_dtypes.bfloat16),
        gate_wT=tr(inputs["moe_gate_w"][0]),
        in_wT_m=tr(inputs["moe_in_w"]), out_wT_m=tr(inputs["moe_out_w"]),
        in_wT_s=tr(inputs["std_in_w"]), out_wT_s=tr(inputs["std_out_w"]),
        w1T=wcast(tr(inputs["moe_w1"])), w2T=wcast(tr(inputs["moe_w2"])),
        fc_wT=tr(inputs["std_fc_w"]), proj_wT=tr(inputs["std_proj_w"]),
        mp0=mp0, mp1=mp1, sp0=sp0, sp1=sp1,
    )
    return shared


def _x_slice_T(x, core):
    # x [S, B, D] -> core slice b-major flat [T, D] padded to TP -> transpose
    xs = np.transpose(x[:, core * BL:(core + 1) * BL, :], (1, 0, 2)).reshape(T, D)
    xp = np.zeros((TP, D), np.float32)
    xp[:T] = xs
    return np.ascontiguousarray(xp.T)


LAST_EXEC_NS = None
N_CORES = 8


def _make_runner(nc):
    """Build a cached jitted 8-core runner (mirrors bass2jax.run_bass_via_pjrt)."""
    import jax
    from jax.experimental.shard_map import shard_map
    from jax.sharding import Mesh, PartitionSpec
    from concourse import mybir as mb
    from concourse.bass2jax import (
        _bass_exec_p, install_neuronx_cc_hook, partition_id_tensor)

    install_neuronx_cc_hook()
    partition_name = nc.partition_id_tensor.name if nc.partition_id_tensor else None
    in_names, out_names, out_avals, zero_shapes = [], [], [], []
    for alloc in nc.m.functions[0].allocations:
        if not isinstance(alloc, mb.MemoryLocationSet):
            continue
        name = alloc.memorylocations[0].name
        if alloc.kind == "ExternalInput":
            if name != partition_name:
                in_names.append(name)
        elif alloc.kind == "ExternalOutput":
            shape = tuple(alloc.tensor_shape)
            dtype = mb.dt.np(alloc.dtype)
            out_names.append(name)
            out_avals.append(jax.core.ShapedArray(shape, dtype))
            zero_shapes.append((shape, dtype))
    n_params, n_outs = len(in_names), len(out_names)
    bind_names = list(in_names) + list(out_names)
    if partition_name is not None:
        bind_names.append(partition_name)
    donate = tuple(range(n_params, n_params + n_outs))

    def _body(*args):
        operands = list(args)
        if partition_name is not None:
            operands.append(partition_id_tensor())
        outs = _bass_exec_p.bind(
            *operands,
            out_avals=tuple(out_avals),
            in_names=tuple(bind_names),
            out_names=tuple(out_names),
            lowering_input_output_aliases=(),
            sim_require_finite=True,
            sim_require_nnan=True,
            nc=nc,
        )
        return tuple(outs)

    devices = jax.devices()[:N_CORES]
    mesh = Mesh(np.asarray(devices), ("core",))
    in_specs = (PartitionSpec("core"),) * (n_params + n_outs)
    out_specs = (PartitionSpec("core"),) * n_outs
    sharded = jax.jit(
        shard_map(_body, mesh=mesh, in_specs=in_specs, out_specs=out_specs,
                  check_rep=False),
        donate_argnums=donate, keep_unused=True)

    import jax.numpy as jnp
    from jax.sharding import NamedSharding
    zsharding = NamedSharding(mesh, PartitionSpec("core"))

    @jax.jit
    def _mk_zeros():
        return tuple(
            jnp.zeros((N_CORES * s[0], *s[1:]), d) for (s, d) in zero_shapes)
    mk_zeros = jax.jit(_mk_zeros, out_shardings=tuple(zsharding for _ in zero_shapes))

    def dev_zeros():
        out = mk_zeros()
        jax.block_until_ready(out)
        return list(out)

    return dict(sharded=sharded, in_names=in_names, out_names=out_names,
                dev_zeros=dev_zeros, mesh=mesh, zsharding=zsharding)


def _get_runner():
    if "runner" not in _CACHE:
        _CACHE["nc"] = build_program()
        _CACHE["runner"] = _make_runner(_CACHE["nc"])
    return _CACHE["runner"]


def _device_inputs(inputs):
    import jax
    shared = _prep_core_inputs(inputs, 0)
    in_maps = []
    for core in range(N_CORES):
        m = dict(shared)
        m["xT"] = _x_slice_T(inputs["x"], core)
        in_maps.append(m)
    r = _get_runner()
    concat = [
        np.concatenate([np.asarray(m[name]) for m in in_maps], axis=0)
        for name in r["in_names"]
    ]
    return [jax.device_put(a, r["zsharding"]) for a in concat]


def _run(dev_in):
    import jax
    r = _get_runner()
    out = r["sharded"](*dev_in, *r["dev_zeros"]())
    jax.block_until_ready(out)
    return out


def _assemble(out_arrs):
    r = _get_runner()
    by_name = {}
    for i, name in enumerate(r["out_names"]):
        a = np.asarray(out_arrs[i])
        by_name[name] = a.reshape(N_CORES, a.shape[0] // N_CORES, *a.shape[1:])
    y = np.zeros((S, B, D), np.float32)
    fl = np.zeros((S * B, E), np.float32)
    srange = np.arange(S) * B
    for core in range(N_CORES):
        yl = by_name["yT"][core].T.reshape(BL, S, D)
        y[:, core * BL:(core + 1) * BL, :] = np.transpose(yl, (1, 0, 2))
        lg = by_name["logits_out"][core][:T].reshape(BL, S, E)
        for b in range(BL):
            fl[srange + core * BL + b] = lg[b]
    return y, fl


def kernel(**inputs):
    global LAST_EXEC_NS
    inputs = {k: np.asarray(v, dtype=np.float32) for k, v in inputs.items()}
    dev_in = _device_inputs(inputs)
    t0 = time.time()
    out = _run(dev_in)
    t1 = time.time()
    LAST_EXEC_NS = int((t1 - t0) * 1e9)
    return _assemble(out)
